# revision 33
# baseline (speedup 1.0000x reference)
"""Trainium2 Bass kernel for nn_Net_48301202211072 (GNN message passing).

2-layer GraphConv + TopKPooling + readout + MLP head, sharded over 8
NeuronCores. Strategy (v2):

- Nodes (and incident edges, grouped by destination) are sharded across
  cores. Edges are packed column-major into 11 destination bins per core
  (<=128 nodes and <=2048 edges per bin), 2 blocks of 1024 edge slots
  per bin. Everything is bf16 except score/threshold arithmetic.
- Layer-1 aggregation: the host pre-arranges per-edge source rows (xe)
  and edge-weighted one-hot matrices (Woh); the device streams both and
  aggregates with plain PE matmuls (no dma_gather, no per-edge DVE).
- Layer-2 aggregation: each node's table row [h1 | z1] is AllGather'd
  right after conv1 (before topk), so the big collective and the tau1
  histogram overlap; dma_gather fetches rows per edge and the per-slot
  scale tanh(z/||p||)*(z>=tau)*w is applied via the one-hot weights.
- TopK threshold: replicated 4-stage 64-bin histogram over the
  AllGather'd scores (bf16 compares, exact-enough within tolerance).
- Readout: ones-matmul mean + masked-transpose max, combined via small
  AllGathers (ro1 rides in the z2 AllGather payload).
- Head: lin1 replicated, lin2 sharded by rows, lin3 by contraction,
  one final AllReduce.
"""
import math
import sys

import numpy as np
import ml_dtypes

sys.path.insert(0, "/opt/trn_rl_repo")

import concourse.bacc as bacc  # noqa: E402
import concourse.mybir as mybir  # noqa: E402
import concourse.tile as tile  # noqa: E402
from concourse import bass_utils  # noqa: E402

FP32 = mybir.dt.float32
BF16 = mybir.dt.bfloat16
I16 = mybir.dt.int16
AX = mybir.AxisListType
OP = mybir.AluOpType
ACT = mybir.ActivationFunctionType
BFNP = ml_dtypes.bfloat16

NCORES = 8
N = 10000
FIN = 256
HID = 500
HPAD = 512
NOUT = 100
NPC = N // NCORES          # 1250 nodes per core
NCH = 11                   # dst bins per core (<=128 nodes, <=2048 edges)
NB = 2                     # blocks of 1024 edge slots per bin
BTOT = NCH * NB            # 22
NPAD = NCH * 128           # 1408 table rows per core
NROWS = NCORES * NPAD      # 11264
ROWB = 640                 # table row: 512 h bf16 + 2 z-as-bf16 + 126 pad
NBINS = 32
NSTAGES = 3
K1 = N // 2
K2 = N // 4
ZRO = NPAD + 1024          # 2432: zm2 + ro1 payload rows per core
SPLITB = 6                 # bins 0..5 AllGather'd early (rows 0:768)
SPLITR = SPLITB * 128      # 768
ASZ = NCORES * SPLITR      # 6144 rows in part A of the gathered table
BIG = 1e30


# ---------------------------------------------------------------------------
# host preprocessing
# ---------------------------------------------------------------------------

def _pack(x, edge_src, edge_dst, edge_weight):
    src = np.asarray(edge_src, np.int64)
    dst = np.asarray(edge_dst, np.int64)
    w = np.asarray(edge_weight, np.float32)
    x_bf = np.ascontiguousarray(np.asarray(x, np.float32)).astype(BFNP)

    # pass 1: per-core greedy bin boundaries + node->table-row map
    binrow = np.zeros((NCORES, NPC), np.int64)   # local node -> row in [0,NPAD)
    starts_all, counts_all = [], []
    for c in range(NCORES):
        lo = c * NPC
        m = (dst >= lo) & (dst < lo + NPC)
        ed = dst[m] - lo
        deg = np.bincount(ed, minlength=NPC)
        bstart, bnodes = [], []
        n0 = 0
        while n0 < NPC:
            e_acc, nn = 0, 0
            while n0 + nn < NPC and nn < 128 and e_acc + deg[n0 + nn] <= 2048:
                e_acc += deg[n0 + nn]
                nn += 1
            assert nn > 0
            bstart.append(n0)
            bnodes.append(nn)
            n0 += nn
        assert len(bstart) <= NCH, f"core {c} needs {len(bstart)} bins"
        while len(bstart) < NCH:
            bstart.append(NPC)
            bnodes.append(0)
        bstart = np.asarray(bstart, np.int64)
        bnodes = np.asarray(bnodes, np.int64)
        for b in range(NCH):
            s, nn = bstart[b], bnodes[b]
            binrow[c, s:s + nn] = b * 128 + np.arange(nn)
        starts_all.append(bstart)
        counts_all.append(bnodes)

    per_core = []
    for c in range(NCORES):
        lo = c * NPC
        m = (dst >= lo) & (dst < lo + NPC)
        es, ed, ew = src[m], dst[m] - lo, w[m]
        order = np.argsort(ed, kind="stable")
        es, ed, ew = es[order], ed[order], ew[order]
        bstart, bnodes = starts_all[c], counts_all[c]
        # edge ranges per bin (bins are consecutive node ranges)
        bin_edge_start = np.searchsorted(ed, bstart)
        bin_edge_end = np.searchsorted(ed, bstart + bnodes)

        # slot assignment (column-major within each bin's 2 blocks)
        srcslot = np.full(BTOT * 1024, -1, np.int64)
        dslot = np.zeros(BTOT * 1024, np.int64)
        wslot = np.zeros(BTOT * 1024, np.float32)
        for b in range(NCH):
            e0, e1 = bin_edge_start[b], bin_edge_end[b]
            cnt = e1 - e0
            assert cnt <= NB * 1024
            u = np.arange(cnt)
            blk = b * NB + u // 1024
            u2 = u % 1024
            pos = blk * 1024 + u2
            srcslot[pos] = es[e0:e1]
            dslot[pos] = ed[e0:e1] - bstart[b]
            wslot[pos] = ew[e0:e1]

        real = srcslot >= 0
        # xe: [128, BTOT*8*256] pre-gathered source rows, bf16
        rows = np.zeros((BTOT * 1024, FIN), BFNP)
        rows[real] = x_bf[srcslot[real]]
        xe = np.ascontiguousarray(
            rows.reshape(BTOT, 8, 128, FIN).transpose(2, 0, 1, 3)
            .reshape(128, BTOT * 8 * FIN))

        # woh: [128, BTOT*8*128] edge-weighted one-hots, bf16
        woh = np.zeros((128, BTOT * 8 * 128), np.float32)
        pos = np.nonzero(real)[0]
        blk = pos // 1024
        u2 = pos % 1024
        p = u2 % 128
        j = u2 // 128
        woh[p, (blk * 8 + j) * 128 + dslot[pos]] = wslot[pos]
        woh = woh.astype(BFNP)

        # idx2: slot -> row in the AllGather'd table
        sc = srcslot // NPC
        slo = srcslot - sc * NPC
        idx2 = np.zeros(BTOT * 1024, np.int64)
        idx2[real] = sc[real] * NPAD + binrow[sc[real], slo[real]]
        per_core.append(dict(xe=xe, woh=woh, idx2=idx2,
                             bstart=bstart, bnodes=bnodes))
    return per_core


def _wrap16(idx_flat):
    """[BTOT*1024] -> [128, BTOT*64] int16, per-block wrapped-16 replicated."""
    out = np.zeros((128, BTOT * 64), np.int16)
    for blk in range(BTOT):
        b = idx_flat[blk * 1024:(blk + 1) * 1024].astype(np.int16)
        t = b.reshape(64, 16).T          # [16, 64]
        out[:, blk * 64:(blk + 1) * 64] = np.tile(t, (8, 1))
    return out


def _host_inputs(inputs, prep):
    x = np.ascontiguousarray(np.asarray(inputs["x"], np.float32))

    def padT(a, rows, cols):
        out = np.zeros((rows, cols), np.float32)
        t = np.asarray(a, np.float32).T
        out[: t.shape[0], : t.shape[1]] = t
        return out.astype(BFNP)

    w1relT = padT(inputs["W1_rel"], FIN, HPAD)
    w1rootT = padT(inputs["W1_root"], FIN, HPAD)
    w2relT = padT(inputs["W2_rel"], HPAD, HPAD)
    w2rootT = padT(inputs["W2_root"], HPAD, HPAD)

    def rowv(v):
        out = np.zeros((1, HPAD), np.float32)
        vv = np.asarray(v, np.float32)
        out[0, : vv.shape[0]] = vv
        return out

    b1row = rowv(inputs["b1"]).astype(BFNP)
    b2row = rowv(inputs["b2"]).astype(BFNP)
    p1f = rowv(inputs["p1_w"])
    p2f = rowv(inputs["p2_w"])
    p1r = np.tile(p1f, (128, 1)).astype(BFNP)
    p2r = np.tile(p2f, (128, 1)).astype(BFNP)

    iotaB = np.tile(np.arange(NBINS, dtype=np.float32)[None, :], (128, 1))
    identb = np.eye(128, dtype=np.float32).astype(BFNP)
    identf = np.eye(128, dtype=np.float32)
    ones128f = np.ones((1, 128), np.float32)
    ones1b = np.ones((1, 128), np.float32).astype(BFNP)
    onesPb = np.ones((128, 1), np.float32).astype(BFNP)
    ones11 = np.ones((1, 1), np.float32).astype(BFNP)

    # mask of z positions inside the flattened z2ro AllGather payload
    g = np.arange(128 * (NCORES * ZRO // 128), dtype=np.int64)
    romask = ((g % ZRO) < NPAD).astype(np.float32).reshape(
        128, NCORES * ZRO // 128)

    lin1W = np.asarray(inputs["lin1_W"], np.float32)   # [2000, 1000]
    lin2W = np.asarray(inputs["lin2_W"], np.float32)   # [4000, 2000]
    lin3W = np.asarray(inputs["lin3_W"], np.float32)   # [100, 4000]
    lin1b = np.asarray(inputs["lin1_b"], np.float32)
    lin2b = np.asarray(inputs["lin2_b"], np.float32)
    lin3b = np.asarray(inputs["lin3_b"], np.float32)

    # lin1 replicated: rows = z layout [max 0:500 | pad | mean 512:1012 | pad]
    l1T = np.zeros((1024, 2048), np.float32)
    sh = lin1W.T                                       # [1000, 2000]
    l1T[:500, :2000] = sh[:500]
    l1T[512:1012, :2000] = sh[500:]
    l1T = l1T.astype(BFNP)
    b1h = np.zeros((1, 2048), np.float32)
    b1h[0, :2000] = lin1b
    b1h = b1h.astype(BFNP)

    per_core = []
    for c in range(NCORES):
        pr = prep[c]
        bstart, bnodes = pr["bstart"], pr["bnodes"]

        xT = np.zeros((FIN, NPAD), np.float32)
        padmask = np.zeros((128, NCH), np.float32)
        for b in range(NCH):
            s, nn = bstart[b], bnodes[b]
            if nn:
                xT[:, b * 128: b * 128 + nn] = x[c * NPC + s: c * NPC + s + nn].T
                padmask[:nn, b] = 1.0
        xT = xT.astype(BFNP)

        l2T = np.zeros((2048, 500), np.float32)
        l2T[:2000] = lin2W[c * 500:(c + 1) * 500].T
        l2T = l2T.astype(BFNP)
        b2h = np.zeros((1, 512), np.float32)
        b2h[0, :500] = lin2b[c * 500:(c + 1) * 500]
        b2h = b2h.astype(BFNP)

        l3T = np.zeros((512, 128), np.float32)
        l3T[:500, :NOUT] = lin3W[:, c * 500:(c + 1) * 500].T
        l3T = l3T.astype(BFNP)
        b3h = np.zeros((128, 1), np.float32)
        b3h[:NOUT, 0] = lin3b

        per_core.append(dict(
            xe=pr["xe"], woh=pr["woh"], idx2=_wrap16(pr["idx2"]),
            padmask=padmask, xT=xT,
            w1relT=w1relT, w1rootT=w1rootT, w2relT=w2relT, w2rootT=w2rootT,
            b1row=b1row, b2row=b2row, p1f=p1f, p2f=p2f, p1r=p1r, p2r=p2r,
            iotaB=iotaB, identb=identb, identf=identf, ones128f=ones128f,
            ones1b=ones1b, onesPb=onesPb, ones11=ones11,
            romask=romask,
            l1T=l1T, b1h=b1h, l2T=l2T, b2h=b2h, l3T=l3T, b3h=b3h,
        ))
    return per_core


# ---------------------------------------------------------------------------
# device program
# ---------------------------------------------------------------------------

def _build():
    nc = bacc.Bacc("TRN2", target_bir_lowering=False, debug=False,
                   num_devices=NCORES)

    def din(name, shape, dt=FP32):
        return nc.dram_tensor(name, shape, dt, kind="ExternalInput")

    xe = din("xe", [128, BTOT * 8 * FIN], BF16)
    woh = din("woh", [128, BTOT * 8 * 128], BF16)
    idx2 = din("idx2", [128, BTOT * 64], I16)
    padmask = din("padmask", [128, NCH])
    xT = din("xT", [FIN, NPAD], BF16)
    w1relT = din("w1relT", [FIN, HPAD], BF16)
    w1rootT = din("w1rootT", [FIN, HPAD], BF16)
    w2relT = din("w2relT", [HPAD, HPAD], BF16)
    w2rootT = din("w2rootT", [HPAD, HPAD], BF16)
    b1row = din("b1row", [1, HPAD], BF16)
    b2row = din("b2row", [1, HPAD], BF16)
    p1f = din("p1f", [1, HPAD])
    p2f = din("p2f", [1, HPAD])
    p1r = din("p1r", [128, HPAD], BF16)
    p2r = din("p2r", [128, HPAD], BF16)
    iotaB = din("iotaB", [128, NBINS])
    identb = din("identb", [128, 128], BF16)
    identf = din("identf", [128, 128])
    ones128f = din("ones128f", [1, 128])
    ones1b = din("ones1b", [1, 128], BF16)
    onesPb = din("onesPb", [128, 1], BF16)
    ones11 = din("ones11", [1, 1], BF16)
    romask = din("romask", [128, NCORES * ZRO // 128])
    l1T = din("l1T", [1024, 2048], BF16)
    b1h = din("b1h", [1, 2048], BF16)
    l2T = din("l2T", [2048, 500], BF16)
    b2h = din("b2h", [1, 512], BF16)
    l3T = din("l3T", [512, 128], BF16)
    b3h = din("b3h", [128, 1])

    out = nc.dram_tensor("out", [1, NOUT], FP32, kind="ExternalOutput")

    RG = [list(range(NCORES))]

    with tile.TileContext(nc) as tc:
        with (
            tc.tile_pool(name="const", bufs=1) as cp,
            tc.tile_pool(name="stream", bufs=3) as sp,
            tc.tile_pool(name="gather", bufs=2) as gp,
            tc.tile_pool(name="work", bufs=1) as wp,
            tc.tile_pool(name="big", bufs=1) as bigp,
            tc.tile_pool(name="psA", bufs=2, space="PSUM") as psA,
            tc.tile_pool(name="psB", bufs=2, space="PSUM") as psB,
            tc.tile_pool(name="psT", bufs=2, space="PSUM") as psT,
            tc.tile_pool(name="psS", bufs=1, space="PSUM") as psS,
            tc.tile_pool(name="dram", bufs=1, space="DRAM") as dr,
        ):
            def load(src, dt=FP32, tag=None):
                tl = cp.tile(list(src.shape), dt, tag=tag or src.name)
                nc.sync.dma_start(tl[:], src[:])
                return tl

            idx2_t = load(idx2, I16)
            pad_t = load(padmask)
            iob_t = load(iotaB)
            idb_t = load(identb, BF16)
            idf_t = load(identf)
            ones_t = load(ones128f)
            ones1b_t = load(ones1b, BF16)
            onesPb_t = load(onesPb, BF16)
            ones11_t = load(ones11, BF16)
            b1row_t = load(b1row, BF16)
            b2row_t = load(b2row, BF16)
            p1f_t = load(p1f)
            p2f_t = load(p2f)
            p1r_t = load(p1r, BF16)
            p2r_t = load(p2r, BF16)
            rom_t = load(romask)

            def load_chunks(src, nchunks, cols, tag, dt=BF16):
                ts = []
                for k in range(nchunks):
                    t = cp.tile([128, cols], dt, tag=f"{tag}{k}")
                    nc.sync.dma_start(t[:], src[k * 128:(k + 1) * 128, :cols])
                    ts.append(t)
                return ts

            w1rel_t = load_chunks(w1relT, 2, HPAD, "w1rel")
            w1root_t = load_chunks(w1rootT, 2, HPAD, "w1root")
            w2rel_t = load_chunks(w2relT, 4, HPAD, "w2rel")
            w2root_t = load_chunks(w2rootT, 4, HPAD, "w2root")
            xT_t = load_chunks(xT, 2, NPAD, "xTc")

            # DRAM internal tiles
            tbl = dr.tile([NPAD, ROWB], BF16)
            tblag = dr.tile([NROWS, ROWB], BF16, addr_space="Shared")
            zsh1 = dr.tile([NPAD, 1], FP32)
            zag1 = dr.tile([NROWS, 1], FP32, addr_space="Shared")
            z2ro = dr.tile([ZRO, 1], FP32)
            z2roag = dr.tile([NCORES * ZRO, 1], FP32, addr_space="Shared")
            ro2in = dr.tile([2, HPAD], FP32)
            ro2ag = dr.tile([2 * NCORES, HPAD], FP32, addr_space="Shared")
            oin = dr.tile([128, 1], FP32)
            oar = dr.tile([128, 1], FP32, addr_space="Shared")
            wrm = dr.tile([16, 1], FP32)
            wrmag = dr.tile([16 * NCORES, 1], FP32, addr_space="Shared")

            # collective-stack warmup: absorb first-collective setup cost
            # while layer 1 computes
            wz = wp.tile([16, 1], FP32, tag="wz")
            nc.vector.memset(wz[:], 0.0)
            nc.sync.dma_start(wrm[:], wz[:])
            nc.gpsimd.collective_compute(
                "AllGather", OP.bypass, replica_groups=RG,
                ins=[wrm[:]], outs=[wrmag[:]])

            # -------- norms first (Sqrt table load hides under L1) ---------
            def inv_norm_b(pf_t, lname):
                """[128,1] broadcast of 1/||p||."""
                sq = wp.tile([1, HPAD], FP32, tag="pnsq")
                nc.vector.tensor_tensor(out=sq[:], in0=pf_t[:], in1=pf_t[:],
                                        op=OP.mult)
                n2 = wp.tile([1, 1], FP32, tag="pn2")
                nc.vector.tensor_reduce(out=n2[:], in_=sq[:], op=OP.add,
                                        axis=AX.X)
                nc.scalar.activation(n2[:], n2[:], ACT.Sqrt)
                nc.vector.reciprocal(n2[:], n2[:])
                ib_ps = psS.tile([128, 1], FP32, tag="small")
                nc.tensor.matmul(out=ib_ps[:], lhsT=ones_t[:], rhs=n2[:],
                                 start=True, stop=True)
                ib = wp.tile([128, 1], FP32, tag=f"invbs{lname}")
                nc.vector.tensor_copy(ib[:], ib_ps[:])
                return ib

            inv1b = inv_norm_b(p1f_t, "l1")
            inv2b = inv_norm_b(p2f_t, "l2")

            # ---------------- histogram k-th threshold ---------------------
            NF2 = NCORES * ZRO // 128
            S_big = wp.tile([128, NF2 * NBINS], BF16, tag="Sbig")

            def topk_tau(zt, nfree, k, lname):
                """zt: [128, nfree] fp32 scores (pads/masked = -BIG).
                returns [128,1] tile with the k-th-largest threshold."""
                mm = wp.tile([128, 2], FP32, tag="mm")
                msk = wp.tile([128, nfree], FP32, tag=f"hmsk{lname}")
                nc.vector.tensor_scalar(msk[:], zt[:], -1e29, 2e30, OP.is_lt,
                                        OP.mult)
                nc.vector.tensor_tensor(out=msk[:], in0=msk[:], in1=zt[:],
                                        op=OP.add)
                nc.vector.tensor_reduce(out=mm[:, 0:1], in_=msk[:], op=OP.min,
                                        axis=AX.X)
                nc.vector.tensor_reduce(out=mm[:, 1:2], in_=zt[:], op=OP.max,
                                        axis=AX.X)
                ztb = wp.tile([128, nfree], BF16, tag=f"ztb{lname}")
                nc.vector.tensor_copy(ztb[:], zt[:])
                lw = wp.tile([1, 2], FP32, tag="lw")  # [lo, w]
                mmT = wp.tile([1, 2, 128], FP32, tag="mmTs")
                for col in range(2):
                    mmT_ps = psS.tile([1, 128], FP32, tag="small")
                    nc.tensor.transpose(out=mmT_ps[:], in_=mm[:, col:col + 1],
                                        identity=idf_t[:])
                    nc.vector.tensor_copy(mmT[:, col, :], mmT_ps[:])
                nc.vector.tensor_reduce(out=lw[:, 0:1], in_=mmT[:, 0, :],
                                        op=OP.min, axis=AX.X)
                nc.vector.tensor_reduce(out=lw[:, 1:2], in_=mmT[:, 1, :],
                                        op=OP.max, axis=AX.X)
                nc.vector.tensor_scalar_add(lw[:, 0:1], lw[:, 0:1], -1e-3)
                nc.vector.tensor_scalar_add(lw[:, 1:2], lw[:, 1:2], 1e-3)
                nc.vector.tensor_tensor(out=lw[:, 1:2], in0=lw[:, 1:2],
                                        in1=lw[:, 0:1], op=OP.subtract)
                nc.vector.tensor_scalar_mul(lw[:, 1:2], lw[:, 1:2], 1.0 / NBINS)

                for st in range(NSTAGES):
                    lwb_ps = psS.tile([128, 2], FP32, tag="small")
                    nc.tensor.matmul(out=lwb_ps[:], lhsT=ones_t[:], rhs=lw[:],
                                     start=True, stop=True)
                    lwb = wp.tile([128, 2], FP32, tag="lwbs")
                    nc.vector.tensor_copy(lwb[:], lwb_ps[:])
                    tt = wp.tile([128, NBINS], FP32, tag="tt")
                    nc.vector.tensor_scalar(tt[:], iob_t[:], lwb[:, 1:2],
                                            lwb[:, 0:1], OP.mult, OP.add)
                    ttb = wp.tile([128, NBINS], BF16, tag="ttb")
                    nc.vector.tensor_copy(ttb[:], tt[:])
                    # S[p, j, n]: count-reduce over n is contiguous
                    S = S_big[:, :NBINS * nfree].rearrange(
                        "p (j n) -> p j n", j=NBINS)
                    nc.vector.tensor_tensor(
                        out=S,
                        in0=ztb[:].unsqueeze(1).broadcast_to(
                            [128, NBINS, nfree]),
                        in1=ttb[:].unsqueeze(2).broadcast_to(
                            [128, NBINS, nfree]),
                        op=OP.is_ge)
                    cntp = wp.tile([128, NBINS], BF16, tag="cntp")
                    with nc.allow_low_precision(
                            reason="counts <= nfree are exact in bf16"):
                        nc.vector.tensor_reduce(
                            out=cntp[:], in_=S, op=OP.add, axis=AX.X)
                    cnt_ps = psS.tile([1, NBINS], FP32, tag="small")
                    nc.tensor.matmul(out=cnt_ps[:], lhsT=onesPb_t[:],
                                     rhs=cntp[:], start=True, stop=True)
                    fl = wp.tile([1, NBINS], FP32, tag="fl")
                    nc.vector.tensor_scalar(fl[:], cnt_ps[:], float(k), None,
                                            OP.is_ge)
                    js = wp.tile([1, 1], FP32, tag="js")
                    nc.vector.tensor_reduce(out=js[:], in_=fl[:], op=OP.add,
                                            axis=AX.X)
                    nc.vector.tensor_scalar_add(js[:], js[:], -1.0)
                    nc.vector.tensor_scalar(lw[:, 0:1], js[:], lw[:, 1:2],
                                            lw[:, 0:1], OP.mult, OP.add)
                    if st != NSTAGES - 1:
                        nc.vector.tensor_scalar_mul(lw[:, 1:2], lw[:, 1:2],
                                                    1.0 / NBINS)
                taub_ps = psS.tile([128, 1], FP32, tag="small")
                nc.tensor.matmul(out=taub_ps[:], lhsT=ones_t[:],
                                 rhs=lw[:, 0:1], start=True, stop=True)
                taub = wp.tile([128, 1], FP32, tag=f"taubs{lname}")
                nc.vector.tensor_copy(taub[:], taub_ps[:])
                return taub

            # ======================= layer 1 ===============================
            h1 = bigp.tile([128, NCH * HPAD], BF16, tag="h1_all")
            z1 = wp.tile([128, NCH], FP32, tag="z1")
            # aggT tiles shared between layers (L1 uses the first two; its
            # dense reads complete before L2 overwrites them)
            aggT2 = [bigp.tile([128, NPAD], BF16, tag=f"aggT2_{fc}",
                               name=f"aggT2_{fc}")
                     for fc in range(4)]
            aggT1 = aggT2[:2]

            for b in range(NCH):
                agg_ps = psA.tile([128, HPAD], FP32, tag="aggps")
                for k in range(NB):
                    B = b * NB + k
                    xeb = sp.tile([128, 8, FIN], BF16, tag="xeb")
                    nc.sync.dma_start(
                        xeb[:].rearrange("p a f -> p (a f)"),
                        xe[:, B * 8 * FIN:(B + 1) * 8 * FIN])
                    wohb = sp.tile([128, 8, 128], BF16, tag="wohb1")
                    nc.sync.dma_start(
                        wohb[:].rearrange("p a d -> p (a d)"),
                        woh[:, B * 1024:(B + 1) * 1024])
                    for j in range(8):
                        nc.tensor.matmul(
                            out=agg_ps[:, :FIN], lhsT=wohb[:, j, :],
                            rhs=xeb[:, j, :],
                            start=(k == 0 and j == 0),
                            stop=(k == NB - 1 and j == 7))
                # transpose agg -> aggT1 chunks
                aggc = wp.tile([128, FIN], BF16, tag="aggc", bufs=2)
                nc.scalar.activation(aggc[:], agg_ps[:, :FIN], ACT.Copy)
                for fc in range(2):
                    tps = psT.tile([128, 128], BF16, tag="tps")
                    nc.tensor.transpose(out=tps[:],
                                        in_=aggc[:, fc * 128:(fc + 1) * 128],
                                        identity=idb_t[:])
                    nc.scalar.activation(aggT1[fc][:, b * 128:(b + 1) * 128],
                                         tps[:], ACT.Copy)
                # dense: h = relu(b1 + aggT.T @ w1relT + xT.T @ w1rootT)
                hp = psB.tile([128, HPAD], FP32, tag="hps")
                nc.tensor.matmul(out=hp[:], lhsT=ones1b_t[:], rhs=b1row_t[:],
                                 start=True, stop=False)
                for fc in range(2):
                    nc.tensor.matmul(
                        out=hp[:], lhsT=aggT1[fc][:, b * 128:(b + 1) * 128],
                        rhs=w1rel_t[fc][:], start=False, stop=False)
                for fc in range(2):
                    nc.tensor.matmul(
                        out=hp[:], lhsT=xT_t[fc][:, b * 128:(b + 1) * 128],
                        rhs=w1root_t[fc][:], start=False, stop=(fc == 1))
                hc = h1[:, b * HPAD:(b + 1) * HPAD]
                nc.scalar.activation(hc, hp[:], ACT.Relu)
                # z score (fp32)
                scr = wp.tile([128, HPAD], FP32, tag="scr", bufs=2)
                nc.vector.tensor_tensor(out=scr[:], in0=hc, in1=p1r_t[:],
                                        op=OP.mult)
                nc.vector.tensor_reduce(out=z1[:, b:b + 1], in_=scr[:],
                                        op=OP.add, axis=AX.X)
                # table row: [h | z | pad]
                tblb = wp.tile([128, ROWB], BF16, tag="tblb", bufs=2)
                nc.scalar.activation(tblb[:, 0:HPAD], hp[:], ACT.Relu)
                nc.vector.tensor_copy(
                    tblb[:, HPAD:HPAD + 2].bitcast(FP32), z1[:, b:b + 1])
                nc.sync.dma_start(tbl[b * 128:(b + 1) * 128, :], tblb[:])

            # masked z for selection
            pm30 = wp.tile([128, NCH], FP32, tag="pm30")
            nc.vector.tensor_scalar(pm30[:], pad_t[:], 1.0, BIG, OP.subtract,
                                    OP.mult)
            zm1 = wp.tile([128, NCH], FP32, tag="zm1")
            nc.vector.tensor_tensor(out=zm1[:], in0=z1[:], in1=pad_t[:],
                                    op=OP.mult)
            nc.vector.tensor_tensor(out=zm1[:], in0=zm1[:], in1=pm30[:],
                                    op=OP.add)
            nc.sync.dma_start(
                zsh1[:].rearrange("(b p) o -> p (b o)", p=128), zm1[:])
            nc.gpsimd.collective_compute(
                "AllGather", OP.bypass, replica_groups=RG,
                ins=[tbl[:]], outs=[tblag[:]])
            nc.gpsimd.collective_compute(
                "AllGather", OP.bypass, replica_groups=RG,
                ins=[zsh1[:]], outs=[zag1[:]])

            zt1 = wp.tile([128, NROWS // 128], FP32, tag="zt1")
            nc.sync.dma_start(
                zt1[:], zag1[:].rearrange("(p f) o -> p (f o)", p=128))
            tau1b = topk_tau(zt1, NROWS // 128, K1, "l1")

            # a1 per local bin + kept masks
            kp1 = wp.tile([128, NCH], FP32, tag="kp1")
            nc.vector.tensor_scalar(kp1[:], zm1[:], tau1b[:, 0:1], None,
                                    OP.is_ge)
            s1 = wp.tile([128, NCH], FP32, tag="s1")
            nc.scalar.activation(s1[:], z1[:], ACT.Tanh, scale=inv1b[:, 0:1])
            a1 = wp.tile([128, NCH], FP32, tag="a1")
            nc.vector.tensor_tensor(out=a1[:], in0=s1[:], in1=kp1[:],
                                    op=OP.mult)
            km30 = wp.tile([128, NCH], FP32, tag="km30")
            nc.vector.tensor_scalar(km30[:], kp1[:], 1.0, BIG, OP.subtract,
                                    OP.mult)

            # g1 (scaled, masked transpose) + readout 1
            gmT1 = [bigp.tile([128, NPAD], BF16, tag=f"gmT1_{fc}",
                              name=f"gmT1_{fc}")
                    for fc in range(4)]
            ro1s_ps = psS.tile([1, HPAD], FP32, tag="rosum")
            for b in range(NCH):
                hc = h1[:, b * HPAD:(b + 1) * HPAD]
                g1c = wp.tile([128, HPAD], BF16, tag="g1c", bufs=2)
                nc.vector.tensor_scalar(g1c[:], hc, a1[:, b:b + 1], None,
                                        OP.mult)
                nc.tensor.matmul(out=ro1s_ps[:], lhsT=onesPb_t[:], rhs=g1c[:],
                                 start=(b == 0), stop=(b == NCH - 1))
                gmc = wp.tile([128, HPAD], BF16, tag="gmc", bufs=2)
                nc.vector.tensor_scalar(gmc[:], hc, a1[:, b:b + 1],
                                        km30[:, b:b + 1], OP.mult, OP.add)
                for fc in range(4):
                    tps = psT.tile([128, 128], BF16, tag="tps")
                    nc.tensor.transpose(out=tps[:],
                                        in_=gmc[:, fc * 128:(fc + 1) * 128],
                                        identity=idb_t[:])
                    nc.scalar.activation(gmT1[fc][:, b * 128:(b + 1) * 128],
                                         tps[:], ACT.Copy)
            m1T = wp.tile([128, 4], FP32, tag="m1T")
            for fc in range(4):
                nc.vector.tensor_reduce(out=m1T[:, fc:fc + 1], in_=gmT1[fc][:],
                                        op=OP.max, axis=AX.X)
            ro1s = wp.tile([1, HPAD], FP32, tag="ro1s")
            nc.vector.tensor_copy(ro1s[:], ro1s_ps[:])
            # ro1 rides in the z2ro payload (rows NPAD.. and NPAD+512..)
            nc.sync.dma_start(z2ro[NPAD:NPAD + HPAD, :]
                              .rearrange("f o -> o f"), ro1s[:])
            nc.sync.dma_start(
                z2ro[NPAD + HPAD:NPAD + 1024, :]
                .rearrange("(c p) o -> p (c o)", p=128), m1T[:])

            # ======================= layer 2 ===============================
            h2 = bigp.tile([128, NCH * HPAD], BF16, tag="h2_all")
            z2 = wp.tile([128, NCH], FP32, tag="z2")

            for b in range(NCH):
                agg_ps = psA.tile([128, HPAD], FP32, tag="aggps")
                for k in range(NB):
                    B = b * NB + k
                    gt = gp.tile([128, 8, ROWB], BF16, tag="gath", bufs=2)
                    nc.gpsimd.dma_gather(
                        gt[:], tblag[:], idx2_t[:, B * 64:(B + 1) * 64],
                        1024, 1024, ROWB)
                    wohb = sp.tile([128, 8, 128], BF16, tag="wohb2", bufs=10)
                    nc.sync.dma_start(
                        wohb[:].rearrange("p a d -> p (a d)"),
                        woh[:, B * 1024:(B + 1) * 1024])
                    # per-slot scale a1 = tanh(z*inv)*(z>=tau)
                    zg = gt[:, :, HPAD:HPAD + 2].bitcast(FP32) \
                        .rearrange("p a o -> p (a o)")
                    kp8 = wp.tile([128, 8], FP32, tag="kp8", bufs=2)
                    nc.vector.tensor_scalar(kp8[:], zg, tau1b[:, 0:1], None,
                                            OP.is_ge)
                    s8 = wp.tile([128, 8], FP32, tag="s8", bufs=2)
                    nc.scalar.activation(s8[:], zg, ACT.Tanh,
                                         scale=inv1b[:, 0:1])
                    a1s = wp.tile([128, 8], BF16, tag="a1s", bufs=2)
                    nc.vector.tensor_tensor(out=a1s[:], in0=s8[:], in1=kp8[:],
                                            op=OP.mult)
                    ohs = wp.tile([128, 8, 128], BF16, tag="ohs", bufs=2)
                    nc.vector.tensor_tensor(
                        out=ohs[:], in0=wohb[:],
                        in1=a1s[:].unsqueeze(2).broadcast_to([128, 8, 128]),
                        op=OP.mult)
                    for j in range(8):
                        nc.tensor.matmul(
                            out=agg_ps[:], lhsT=ohs[:, j, :],
                            rhs=gt[:, j, 0:HPAD],
                            start=(k == 0 and j == 0),
                            stop=(k == NB - 1 and j == 7))
                aggc = wp.tile([128, HPAD], BF16, tag="aggc2", bufs=2)
                nc.scalar.activation(aggc[:], agg_ps[:], ACT.Copy)
                for fc in range(4):
                    tps = psT.tile([128, 128], BF16, tag="tps")
                    nc.tensor.transpose(out=tps[:],
                                        in_=aggc[:, fc * 128:(fc + 1) * 128],
                                        identity=idb_t[:])
                    nc.scalar.activation(aggT2[fc][:, b * 128:(b + 1) * 128],
                                         tps[:], ACT.Copy)
                hp = psB.tile([128, HPAD], FP32, tag="hps")
                nc.tensor.matmul(out=hp[:], lhsT=ones1b_t[:], rhs=b2row_t[:],
                                 start=True, stop=False)
                for fc in range(4):
                    nc.tensor.matmul(
                        out=hp[:], lhsT=aggT2[fc][:, b * 128:(b + 1) * 128],
                        rhs=w2rel_t[fc][:], start=False, stop=False)
                for fc in range(4):
                    nc.tensor.matmul(
                        out=hp[:], lhsT=gmT1[fc][:, b * 128:(b + 1) * 128],
                        rhs=w2root_t[fc][:], start=False, stop=(fc == 3))
                hc = h2[:, b * HPAD:(b + 1) * HPAD]
                nc.scalar.activation(hc, hp[:], ACT.Relu)
                scr = wp.tile([128, HPAD], FP32, tag="scr", bufs=2)
                nc.vector.tensor_tensor(out=scr[:], in0=hc, in1=p2r_t[:],
                                        op=OP.mult)
                nc.vector.tensor_reduce(out=z2[:, b:b + 1], in_=scr[:],
                                        op=OP.add, axis=AX.X)

            # masked z2 (kept-in-l1 only) -> z2ro payload -> AllGather
            zm2 = wp.tile([128, NCH], FP32, tag="zm2")
            nc.vector.tensor_tensor(out=zm2[:], in0=z2[:], in1=kp1[:],
                                    op=OP.mult)
            nc.vector.tensor_tensor(out=zm2[:], in0=zm2[:], in1=km30[:],
                                    op=OP.add)
            nc.sync.dma_start(
                z2ro[0:NPAD, :].rearrange("(b p) o -> p (b o)", p=128),
                zm2[:])
            nc.gpsimd.collective_compute(
                "AllGather", OP.bypass, replica_groups=RG,
                ins=[z2ro[:]], outs=[z2roag[:]])

            # tau2 over the masked flat payload
            ztr = wp.tile([128, NF2], FP32, tag="ztr")
            nc.sync.dma_start(
                ztr[:], z2roag[:].rearrange("(p f) o -> p (f o)", p=128))
            zt2 = wp.tile([128, NF2], FP32, tag="zt2")
            nc.vector.tensor_tensor(out=zt2[:], in0=ztr[:], in1=rom_t[:],
                                    op=OP.mult)
            rm30 = wp.tile([128, NF2], FP32, tag="rm30")
            nc.vector.tensor_scalar(rm30[:], rom_t[:], 1.0, BIG, OP.subtract,
                                    OP.mult)
            nc.vector.tensor_tensor(out=zt2[:], in0=zt2[:], in1=rm30[:],
                                    op=OP.add)
            tau2b = topk_tau(zt2, NF2, K2, "l2")

            kp2 = wp.tile([128, NCH], FP32, tag="kp2")
            nc.vector.tensor_scalar(kp2[:], zm2[:], tau2b[:, 0:1], None,
                                    OP.is_ge)
            s2 = wp.tile([128, NCH], FP32, tag="s2")
            nc.scalar.activation(s2[:], z2[:], ACT.Tanh, scale=inv2b[:, 0:1])
            a2 = wp.tile([128, NCH], FP32, tag="a2")
            nc.vector.tensor_tensor(out=a2[:], in0=s2[:], in1=kp2[:],
                                    op=OP.mult)
            km30b = wp.tile([128, NCH], FP32, tag="km30b")
            nc.vector.tensor_scalar(km30b[:], kp2[:], 1.0, BIG, OP.subtract,
                                    OP.mult)

            ro2s_ps = psS.tile([1, HPAD], FP32, tag="rosum")
            m2T = wp.tile([128, 4], FP32, tag="m2T")
            nc.vector.memset(m2T[:], -1e30)
            for b in range(NCH):
                hc = h2[:, b * HPAD:(b + 1) * HPAD]
                g2c = wp.tile([128, HPAD], BF16, tag="g1c", bufs=2)
                nc.vector.tensor_scalar(g2c[:], hc, a2[:, b:b + 1], None,
                                        OP.mult)
                nc.tensor.matmul(out=ro2s_ps[:], lhsT=onesPb_t[:], rhs=g2c[:],
                                 start=(b == 0), stop=(b == NCH - 1))
                gmc = wp.tile([128, HPAD], BF16, tag="gmc", bufs=2)
                nc.vector.tensor_scalar(gmc[:], hc, a2[:, b:b + 1],
                                        km30b[:, b:b + 1], OP.mult, OP.add)
                for fc in range(4):
                    tps = psT.tile([128, 128], BF16, tag="tps")
                    nc.tensor.transpose(out=tps[:],
                                        in_=gmc[:, fc * 128:(fc + 1) * 128],
                                        identity=idb_t[:])
                    red = wp.tile([128, 1], FP32, tag="redm", bufs=2)
                    nc.vector.tensor_reduce(out=red[:], in_=tps[:],
                                            op=OP.max, axis=AX.X)
                    nc.vector.tensor_tensor(out=m2T[:, fc:fc + 1],
                                            in0=m2T[:, fc:fc + 1],
                                            in1=red[:], op=OP.max)
            ro2s = wp.tile([1, HPAD], FP32, tag="ro2s")
            nc.vector.tensor_copy(ro2s[:], ro2s_ps[:])
            nc.sync.dma_start(ro2in[0:1, :], ro2s[:])
            nc.sync.dma_start(
                ro2in[1:2, :].rearrange("o (c p) -> p (o c)", p=128), m2T[:])
            nc.gpsimd.collective_compute(
                "AllGather", OP.bypass, replica_groups=RG,
                ins=[ro2in[:]], outs=[ro2ag[:]])

            # ======================= readout combine + head ================
            # ro1 lives in z2roag rows [s*ZRO+NPAD, s*ZRO+NPAD+1024)
            mx1 = wp.tile([128, 4], FP32, tag="mx1")
            mn1 = wp.tile([128, 4], FP32, tag="mn1")
            sums1 = wp.tile([128, 4, NCORES], FP32, tag="cmb1")
            maxs1 = wp.tile([128, 4, NCORES], FP32, tag="cmbm1")
            for s in range(NCORES):
                base = s * ZRO + NPAD
                nc.sync.dma_start(
                    sums1[:, :, s],
                    z2roag[base:base + HPAD, :]
                    .rearrange("(c p) o -> p (c o)", p=128))
                nc.sync.dma_start(
                    maxs1[:, :, s],
                    z2roag[base + HPAD:base + 1024, :]
                    .rearrange("(c p) o -> p (c o)", p=128))
            nc.vector.tensor_reduce(out=mn1[:], in_=sums1[:], op=OP.add,
                                    axis=AX.X)
            nc.vector.tensor_reduce(out=mx1[:], in_=maxs1[:], op=OP.max,
                                    axis=AX.X)
            nc.vector.tensor_scalar_mul(mn1[:], mn1[:], 1.0 / K1)

            mx2 = wp.tile([128, 4], FP32, tag="mx2")
            mn2 = wp.tile([128, 4], FP32, tag="mn2")
            sums2 = wp.tile([128, 4, 2 * NCORES], FP32, tag="cmb2")
            for r in range(2 * NCORES):
                nc.sync.dma_start(
                    sums2[:, :, r],
                    ro2ag[r:r + 1, :].rearrange("o (c p) -> p (o c)", p=128))
            s_ap = sums2[:].rearrange("p c (s t) -> p c t s", t=2)
            nc.vector.tensor_reduce(out=mn2[:], in_=s_ap[:, :, 0, :],
                                    op=OP.add, axis=AX.X)
            nc.vector.tensor_reduce(out=mx2[:], in_=s_ap[:, :, 1, :],
                                    op=OP.max, axis=AX.X)
            nc.vector.tensor_scalar_mul(mn2[:], mn2[:], 1.0 / K2)

            zT = wp.tile([128, 8], FP32, tag="zT")
            nc.vector.tensor_tensor(out=zT[:, 0:4], in0=mx1[:], in1=mx2[:],
                                    op=OP.add)
            nc.vector.tensor_tensor(out=zT[:, 4:8], in0=mn1[:], in1=mn2[:],
                                    op=OP.add)
            zTb = wp.tile([128, 8], BF16, tag="zTb")
            nc.vector.tensor_copy(zTb[:], zT[:])

            # lin1 replicated: z1h [1, 2048] via psum-row matmuls
            b1h_t = load(b1h, BF16)
            qt = [psA.tile([128, HPAD], FP32, tag="aggps", name=f"hq{q}")
                  if q < 2 else
                  psB.tile([128, HPAD], FP32, tag="hps", name=f"hq{q}")
                  for q in range(4)]
            for q in range(4):
                nc.tensor.matmul(out=qt[q][0:1, :], lhsT=ones11_t[:],
                                 rhs=b1h_t[:, q * 512:(q + 1) * 512],
                                 start=True, stop=False, skip_group_check=True)
            for t in range(8):
                l1c = sp.tile([128, 2048], BF16, tag="l1s")
                nc.sync.dma_start(l1c[:], l1T[t * 128:(t + 1) * 128, :])
                for q in range(4):
                    nc.tensor.matmul(
                        out=qt[q][0:1, :], lhsT=zTb[:, t:t + 1],
                        rhs=l1c[:, q * 512:(q + 1) * 512],
                        start=False, stop=(t == 7), skip_group_check=True)
            z1h = wp.tile([1, 2048], BF16, tag="z1h")
            for q in range(4):
                nc.scalar.activation(z1h[:, q * 512:(q + 1) * 512],
                                     qt[q][0:1, :], ACT.Relu)
            z1hT = wp.tile([128, 16], BF16, tag="z1hT")
            for t in range(16):
                tpv = psT.tile([128, 1], BF16, tag="tps")
                nc.tensor.transpose(out=tpv[:],
                                    in_=z1h[:, t * 128:(t + 1) * 128],
                                    identity=ones11_t[:])
                nc.scalar.activation(z1hT[:, t:t + 1], tpv[:], ACT.Copy)

            # lin2 shard (500 rows), same psum-row form
            b2h_t = load(b2h, BF16)
            o2p = psA.tile([128, HPAD], FP32, tag="aggps")
            nc.tensor.matmul(out=o2p[0:1, 0:512], lhsT=ones11_t[:],
                             rhs=b2h_t[:], start=True, stop=False,
                             skip_group_check=True)
            for t in range(16):
                l2c = sp.tile([128, 500], BF16, tag="l2s")
                nc.sync.dma_start(l2c[:], l2T[t * 128:(t + 1) * 128, :])
                nc.tensor.matmul(out=o2p[0:1, 0:500], lhsT=z1hT[:, t:t + 1],
                                 rhs=l2c[:], start=False, stop=(t == 15),
                                 skip_group_check=True)
            z2h = wp.tile([1, HPAD], BF16, tag="z2h")
            nc.vector.memset(z2h[:], 0.0)
            nc.scalar.activation(z2h[:, 0:500], o2p[0:1, 0:500], ACT.Relu)
            z2hT = wp.tile([128, 4], BF16, tag="z2hT")
            for t in range(4):
                tpv = psT.tile([128, 1], BF16, tag="tps")
                nc.tensor.transpose(out=tpv[:],
                                    in_=z2h[:, t * 128:(t + 1) * 128],
                                    identity=ones11_t[:])
                nc.scalar.activation(z2hT[:, t:t + 1], tpv[:], ACT.Copy)

            # lin3 partial (own contraction shard) + AllReduce
            l3_t = load_chunks(l3T, 4, 128, "l3Tc")
            b3h_t = load(b3h)
            o3p = psB.tile([128, 1], FP32, tag="hps")
            for t in range(4):
                nc.tensor.matmul(out=o3p[:], lhsT=l3_t[t][:],
                                 rhs=z2hT[:, t:t + 1],
                                 start=(t == 0), stop=(t == 3))
            o3 = wp.tile([128, 1], FP32, tag="o3")
            nc.vector.tensor_copy(o3[:], o3p[:])
            nc.sync.dma_start(oin[:], o3[:])
            nc.gpsimd.collective_compute(
                "AllReduce", OP.add, replica_groups=RG,
                ins=[oin[:]], outs=[oar[:]])
            fin = wp.tile([128, 1], FP32, tag="fin")
            nc.sync.dma_start(fin[:], oar[:])
            nc.scalar.activation(fin[:], fin[:], ACT.Sigmoid,
                                 bias=b3h_t[:, 0:1])
            nc.sync.dma_start(out[:].rearrange("o f -> f o"), fin[:NOUT, :])

    nc.compile()
    return nc


# ---------------------------------------------------------------------------
# entry point
# ---------------------------------------------------------------------------

_CACHE = {}
TRACE = False


def kernel(**inputs):
    prep = _pack(inputs["x"], inputs["edge_src"], inputs["edge_dst"],
                 inputs["edge_weight"])
    if "nc" not in _CACHE:
        _CACHE["nc"] = _build()
    nc = _CACHE["nc"]
    in_maps = _host_inputs(inputs, prep)
    res = bass_utils.run_bass_kernel_spmd(
        nc, in_maps, core_ids=list(range(NCORES)), trace=TRACE)
    kernel.last_results = res
    return res.results[0]["out"]


if __name__ == "__main__":
    dat = np.load("/tmp/inputs.npz")
    inputs = {k: dat[k] for k in dat.files}
    got = kernel(**inputs)
    exp = np.load("/tmp/expected.npy")
    err = np.abs(got - exp).max()
    rel = err / np.abs(exp).max()
    print("out[0,:6] =", got[0, :6])
    print("exp[0,:6] =", exp[0, :6])
    print("max abs err:", err, "rel:", rel)


# revision 34
# speedup vs baseline: 1.0899x; 1.0899x over previous
"""Trainium2 Bass kernel for nn_Net_48301202211072 (GNN message passing).

2-layer GraphConv + TopKPooling + readout + MLP head, sharded over 8
NeuronCores. Strategy (v2):

- Nodes (and incident edges, grouped by destination) are sharded across
  cores. Edges are packed column-major into 11 destination bins per core
  (<=128 nodes and <=2048 edges per bin), 2 blocks of 1024 edge slots
  per bin. Everything is bf16 except score/threshold arithmetic.
- Layer-1 aggregation: the host pre-arranges per-edge source rows (xe)
  and edge-weighted one-hot matrices (Woh); the device streams both and
  aggregates with plain PE matmuls (no dma_gather, no per-edge DVE).
- Layer-2 aggregation: each node's table row [h1 | z1] is AllGather'd
  right after conv1 (before topk), so the big collective and the tau1
  histogram overlap; dma_gather fetches rows per edge and the per-slot
  scale tanh(z/||p||)*(z>=tau)*w is applied via the one-hot weights.
- TopK threshold: replicated 4-stage 64-bin histogram over the
  AllGather'd scores (bf16 compares, exact-enough within tolerance).
- Readout: ones-matmul mean + masked-transpose max, combined via small
  AllGathers (ro1 rides in the z2 AllGather payload).
- Head: lin1 replicated, lin2 sharded by rows, lin3 by contraction,
  one final AllReduce.
"""
import math
import sys

import numpy as np
import ml_dtypes

sys.path.insert(0, "/opt/trn_rl_repo")

import concourse.bacc as bacc  # noqa: E402
import concourse.mybir as mybir  # noqa: E402
import concourse.tile as tile  # noqa: E402
from concourse import bass_utils  # noqa: E402

FP32 = mybir.dt.float32
BF16 = mybir.dt.bfloat16
I16 = mybir.dt.int16
AX = mybir.AxisListType
OP = mybir.AluOpType
ACT = mybir.ActivationFunctionType
BFNP = ml_dtypes.bfloat16

NCORES = 8
N = 10000
FIN = 256
HID = 500
HPAD = 512
NOUT = 100
NPC = N // NCORES          # 1250 nodes per core
NCH = 11                   # dst bins per core (<=128 nodes, <=2048 edges)
NB = 2                     # blocks of 1024 edge slots per bin
BTOT = NCH * NB            # 22
NPAD = NCH * 128           # 1408 table rows per core
NROWS = NCORES * NPAD      # 11264
ROWB = 640                 # table row: 512 h bf16 + 2 z-as-bf16 + 126 pad
NBINS = 32
NSTAGES = 3
K1 = N // 2
K2 = N // 4
ZRO = NPAD + 1024          # 2432: zm2 + ro1 payload rows per core
SPLITB = 6                 # bins 0..5 AllGather'd early (rows 0:768)
SPLITR = SPLITB * 128      # 768
ASZ = NCORES * SPLITR      # 6144 rows in part A of the gathered table
BIG = 1e30


# ---------------------------------------------------------------------------
# host preprocessing
# ---------------------------------------------------------------------------

def _pack(x, edge_src, edge_dst, edge_weight):
    src = np.asarray(edge_src, np.int64)
    dst = np.asarray(edge_dst, np.int64)
    w = np.asarray(edge_weight, np.float32)
    x_bf = np.ascontiguousarray(np.asarray(x, np.float32)).astype(BFNP)

    # pass 1: per-core greedy bin boundaries + node->table-row map
    binrow = np.zeros((NCORES, NPC), np.int64)   # local node -> row in [0,NPAD)
    starts_all, counts_all = [], []
    for c in range(NCORES):
        lo = c * NPC
        m = (dst >= lo) & (dst < lo + NPC)
        ed = dst[m] - lo
        deg = np.bincount(ed, minlength=NPC)
        bstart, bnodes = [], []
        n0 = 0
        while n0 < NPC:
            e_acc, nn = 0, 0
            while n0 + nn < NPC and nn < 128 and e_acc + deg[n0 + nn] <= 2048:
                e_acc += deg[n0 + nn]
                nn += 1
            assert nn > 0
            bstart.append(n0)
            bnodes.append(nn)
            n0 += nn
        assert len(bstart) <= NCH, f"core {c} needs {len(bstart)} bins"
        while len(bstart) < NCH:
            bstart.append(NPC)
            bnodes.append(0)
        bstart = np.asarray(bstart, np.int64)
        bnodes = np.asarray(bnodes, np.int64)
        for b in range(NCH):
            s, nn = bstart[b], bnodes[b]
            binrow[c, s:s + nn] = b * 128 + np.arange(nn)
        starts_all.append(bstart)
        counts_all.append(bnodes)

    per_core = []
    for c in range(NCORES):
        lo = c * NPC
        m = (dst >= lo) & (dst < lo + NPC)
        es, ed, ew = src[m], dst[m] - lo, w[m]
        order = np.argsort(ed, kind="stable")
        es, ed, ew = es[order], ed[order], ew[order]
        bstart, bnodes = starts_all[c], counts_all[c]
        # edge ranges per bin (bins are consecutive node ranges)
        bin_edge_start = np.searchsorted(ed, bstart)
        bin_edge_end = np.searchsorted(ed, bstart + bnodes)

        # slot assignment (column-major within each bin's 2 blocks)
        srcslot = np.full(BTOT * 1024, -1, np.int64)
        dslot = np.zeros(BTOT * 1024, np.int64)
        wslot = np.zeros(BTOT * 1024, np.float32)
        for b in range(NCH):
            e0, e1 = bin_edge_start[b], bin_edge_end[b]
            cnt = e1 - e0
            assert cnt <= NB * 1024
            u = np.arange(cnt)
            blk = b * NB + u // 1024
            u2 = u % 1024
            pos = blk * 1024 + u2
            srcslot[pos] = es[e0:e1]
            dslot[pos] = ed[e0:e1] - bstart[b]
            wslot[pos] = ew[e0:e1]

        real = srcslot >= 0
        # xe: [128, BTOT*8*256] pre-gathered source rows, bf16
        rows = np.zeros((BTOT * 1024, FIN), BFNP)
        rows[real] = x_bf[srcslot[real]]
        xe = np.ascontiguousarray(
            rows.reshape(BTOT, 8, 128, FIN).transpose(2, 0, 1, 3)
            .reshape(128, BTOT * 8 * FIN))

        # woh: [128, BTOT*8*128] edge-weighted one-hots, bf16
        woh = np.zeros((128, BTOT * 8 * 128), np.float32)
        pos = np.nonzero(real)[0]
        blk = pos // 1024
        u2 = pos % 1024
        p = u2 % 128
        j = u2 // 128
        woh[p, (blk * 8 + j) * 128 + dslot[pos]] = wslot[pos]
        woh = woh.astype(BFNP)

        # idx2: slot -> row in the AllGather'd table
        sc = srcslot // NPC
        slo = srcslot - sc * NPC
        idx2 = np.zeros(BTOT * 1024, np.int64)
        idx2[real] = sc[real] * NPAD + binrow[sc[real], slo[real]]
        per_core.append(dict(xe=xe, woh=woh, idx2=idx2,
                             bstart=bstart, bnodes=bnodes))
    return per_core


def _wrap16(idx_flat):
    """[BTOT*1024] -> [128, BTOT*64] int16, per-block wrapped-16 replicated."""
    out = np.zeros((128, BTOT * 64), np.int16)
    for blk in range(BTOT):
        b = idx_flat[blk * 1024:(blk + 1) * 1024].astype(np.int16)
        t = b.reshape(64, 16).T          # [16, 64]
        out[:, blk * 64:(blk + 1) * 64] = np.tile(t, (8, 1))
    return out


def _host_inputs(inputs, prep):
    x = np.ascontiguousarray(np.asarray(inputs["x"], np.float32))

    def padT(a, rows, cols):
        out = np.zeros((rows, cols), np.float32)
        t = np.asarray(a, np.float32).T
        out[: t.shape[0], : t.shape[1]] = t
        return out.astype(BFNP)

    w1relT = padT(inputs["W1_rel"], FIN, HPAD)
    w1rootT = padT(inputs["W1_root"], FIN, HPAD)
    w2relT = padT(inputs["W2_rel"], HPAD, HPAD)
    w2rootT = padT(inputs["W2_root"], HPAD, HPAD)

    def rowv(v):
        out = np.zeros((1, HPAD), np.float32)
        vv = np.asarray(v, np.float32)
        out[0, : vv.shape[0]] = vv
        return out

    b1row = rowv(inputs["b1"]).astype(BFNP)
    b2row = rowv(inputs["b2"]).astype(BFNP)
    p1f = rowv(inputs["p1_w"])
    p2f = rowv(inputs["p2_w"])
    p1r = np.tile(p1f, (128, 1)).astype(BFNP)
    p2r = np.tile(p2f, (128, 1)).astype(BFNP)

    iotaB = np.tile(np.arange(NBINS, dtype=np.float32)[None, :], (128, 1))
    identb = np.eye(128, dtype=np.float32).astype(BFNP)
    identf = np.eye(128, dtype=np.float32)
    ones128f = np.ones((1, 128), np.float32)
    ones1b = np.ones((1, 128), np.float32).astype(BFNP)
    onesPb = np.ones((128, 1), np.float32).astype(BFNP)
    ones11 = np.ones((1, 1), np.float32).astype(BFNP)

    # mask of z positions inside the flattened z2ro AllGather payload
    g = np.arange(128 * (NCORES * ZRO // 128), dtype=np.int64)
    romask = ((g % ZRO) < NPAD).astype(np.float32).reshape(
        128, NCORES * ZRO // 128)

    lin1W = np.asarray(inputs["lin1_W"], np.float32)   # [2000, 1000]
    lin2W = np.asarray(inputs["lin2_W"], np.float32)   # [4000, 2000]
    lin3W = np.asarray(inputs["lin3_W"], np.float32)   # [100, 4000]
    lin1b = np.asarray(inputs["lin1_b"], np.float32)
    lin2b = np.asarray(inputs["lin2_b"], np.float32)
    lin3b = np.asarray(inputs["lin3_b"], np.float32)

    # lin1 replicated: rows = z layout [max 0:500 | pad | mean 512:1012 | pad]
    l1T = np.zeros((1024, 2048), np.float32)
    sh = lin1W.T                                       # [1000, 2000]
    l1T[:500, :2000] = sh[:500]
    l1T[512:1012, :2000] = sh[500:]
    l1T = l1T.astype(BFNP)
    b1h = np.zeros((1, 2048), np.float32)
    b1h[0, :2000] = lin1b
    b1h = b1h.astype(BFNP)

    per_core = []
    for c in range(NCORES):
        pr = prep[c]
        bstart, bnodes = pr["bstart"], pr["bnodes"]

        xT = np.zeros((FIN, NPAD), np.float32)
        padmask = np.zeros((128, NCH), np.float32)
        for b in range(NCH):
            s, nn = bstart[b], bnodes[b]
            if nn:
                xT[:, b * 128: b * 128 + nn] = x[c * NPC + s: c * NPC + s + nn].T
                padmask[:nn, b] = 1.0
        xT = xT.astype(BFNP)

        l2T = np.zeros((2048, 500), np.float32)
        l2T[:2000] = lin2W[c * 500:(c + 1) * 500].T
        l2T = l2T.astype(BFNP)
        b2h = np.zeros((1, 512), np.float32)
        b2h[0, :500] = lin2b[c * 500:(c + 1) * 500]
        b2h = b2h.astype(BFNP)

        l3T = np.zeros((512, 128), np.float32)
        l3T[:500, :NOUT] = lin3W[:, c * 500:(c + 1) * 500].T
        l3T = l3T.astype(BFNP)
        b3h = np.zeros((128, 1), np.float32)
        b3h[:NOUT, 0] = lin3b

        per_core.append(dict(
            xe=pr["xe"], woh=pr["woh"], idx2=_wrap16(pr["idx2"]),
            padmask=padmask, xT=xT,
            w1relT=w1relT, w1rootT=w1rootT, w2relT=w2relT, w2rootT=w2rootT,
            b1row=b1row, b2row=b2row, p1f=p1f, p2f=p2f, p1r=p1r, p2r=p2r,
            iotaB=iotaB, identb=identb, identf=identf, ones128f=ones128f,
            ones1b=ones1b, onesPb=onesPb, ones11=ones11,
            romask=romask,
            l1T=l1T, b1h=b1h, l2T=l2T, b2h=b2h, l3T=l3T, b3h=b3h,
        ))
    return per_core


# ---------------------------------------------------------------------------
# device program
# ---------------------------------------------------------------------------

def _build():
    nc = bacc.Bacc("TRN2", target_bir_lowering=False, debug=False,
                   num_devices=NCORES)

    def din(name, shape, dt=FP32):
        return nc.dram_tensor(name, shape, dt, kind="ExternalInput")

    xe = din("xe", [128, BTOT * 8 * FIN], BF16)
    woh = din("woh", [128, BTOT * 8 * 128], BF16)
    idx2 = din("idx2", [128, BTOT * 64], I16)
    padmask = din("padmask", [128, NCH])
    xT = din("xT", [FIN, NPAD], BF16)
    w1relT = din("w1relT", [FIN, HPAD], BF16)
    w1rootT = din("w1rootT", [FIN, HPAD], BF16)
    w2relT = din("w2relT", [HPAD, HPAD], BF16)
    w2rootT = din("w2rootT", [HPAD, HPAD], BF16)
    b1row = din("b1row", [1, HPAD], BF16)
    b2row = din("b2row", [1, HPAD], BF16)
    p1f = din("p1f", [1, HPAD])
    p2f = din("p2f", [1, HPAD])
    p1r = din("p1r", [128, HPAD], BF16)
    p2r = din("p2r", [128, HPAD], BF16)
    iotaB = din("iotaB", [128, NBINS])
    identb = din("identb", [128, 128], BF16)
    identf = din("identf", [128, 128])
    ones128f = din("ones128f", [1, 128])
    ones1b = din("ones1b", [1, 128], BF16)
    onesPb = din("onesPb", [128, 1], BF16)
    ones11 = din("ones11", [1, 1], BF16)
    romask = din("romask", [128, NCORES * ZRO // 128])
    l1T = din("l1T", [1024, 2048], BF16)
    b1h = din("b1h", [1, 2048], BF16)
    l2T = din("l2T", [2048, 500], BF16)
    b2h = din("b2h", [1, 512], BF16)
    l3T = din("l3T", [512, 128], BF16)
    b3h = din("b3h", [128, 1])

    out = nc.dram_tensor("out", [1, NOUT], FP32, kind="ExternalOutput")

    RG = [list(range(NCORES))]

    with tile.TileContext(nc) as tc:
        with (
            tc.tile_pool(name="const", bufs=1) as cp,
            tc.tile_pool(name="stream", bufs=3) as sp,
            tc.tile_pool(name="gather", bufs=2) as gp,
            tc.tile_pool(name="work", bufs=1) as wp,
            tc.tile_pool(name="big", bufs=1) as bigp,
            tc.tile_pool(name="psA", bufs=2, space="PSUM") as psA,
            tc.tile_pool(name="psB", bufs=2, space="PSUM") as psB,
            tc.tile_pool(name="psT", bufs=2, space="PSUM") as psT,
            tc.tile_pool(name="psS", bufs=1, space="PSUM") as psS,
            tc.tile_pool(name="dram", bufs=1, space="DRAM") as dr,
        ):
            def load(src, dt=FP32, tag=None):
                tl = cp.tile(list(src.shape), dt, tag=tag or src.name)
                nc.sync.dma_start(tl[:], src[:])
                return tl

            idx2_t = load(idx2, I16)
            pad_t = load(padmask)
            iob_t = load(iotaB)
            idb_t = load(identb, BF16)
            idf_t = load(identf)
            ones_t = load(ones128f)
            ones1b_t = load(ones1b, BF16)
            onesPb_t = load(onesPb, BF16)
            ones11_t = load(ones11, BF16)
            b1row_t = load(b1row, BF16)
            b2row_t = load(b2row, BF16)
            p1f_t = load(p1f)
            p2f_t = load(p2f)
            p1r_t = load(p1r, BF16)
            p2r_t = load(p2r, BF16)
            rom_t = load(romask)

            def load_chunks(src, nchunks, cols, tag, dt=BF16):
                ts = []
                for k in range(nchunks):
                    t = cp.tile([128, cols], dt, tag=f"{tag}{k}")
                    nc.sync.dma_start(t[:], src[k * 128:(k + 1) * 128, :cols])
                    ts.append(t)
                return ts

            w1rel_t = load_chunks(w1relT, 2, HPAD, "w1rel")
            w1root_t = load_chunks(w1rootT, 2, HPAD, "w1root")
            w2rel_t = load_chunks(w2relT, 4, HPAD, "w2rel")
            w2root_t = load_chunks(w2rootT, 4, HPAD, "w2root")
            xT_t = load_chunks(xT, 2, NPAD, "xTc")

            # DRAM internal tiles
            tbl = dr.tile([NPAD, ROWB], BF16)
            tblag = dr.tile([NROWS, ROWB], BF16, addr_space="Shared")
            zsh1 = dr.tile([NPAD, 1], FP32)
            zag1 = dr.tile([NROWS, 1], FP32, addr_space="Shared")
            z2ro = dr.tile([ZRO, 1], FP32)
            z2roag = dr.tile([NCORES * ZRO, 1], FP32, addr_space="Shared")
            ro2in = dr.tile([2, HPAD], FP32)
            ro2ag = dr.tile([2 * NCORES, HPAD], FP32, addr_space="Shared")
            oin = dr.tile([128, 1], FP32)
            oar = dr.tile([128, 1], FP32, addr_space="Shared")
            wrm = dr.tile([16, 1], FP32)
            wrmag = dr.tile([16 * NCORES, 1], FP32, addr_space="Shared")

            # collective-stack warmup: absorb first-collective setup cost
            # while layer 1 computes
            wz = wp.tile([16, 1], FP32, tag="wz")
            nc.vector.memset(wz[:], 0.0)
            nc.sync.dma_start(wrm[:], wz[:])
            nc.gpsimd.collective_compute(
                "AllGather", OP.bypass, replica_groups=RG,
                ins=[wrm[:]], outs=[wrmag[:]])

            # -------- norms first (Sqrt table load hides under L1) ---------
            def inv_norm_b(pf_t, lname):
                """[128,1] broadcast of 1/||p||."""
                sq = wp.tile([1, HPAD], FP32, tag="pnsq")
                nc.vector.tensor_tensor(out=sq[:], in0=pf_t[:], in1=pf_t[:],
                                        op=OP.mult)
                n2 = wp.tile([1, 1], FP32, tag="pn2")
                nc.vector.tensor_reduce(out=n2[:], in_=sq[:], op=OP.add,
                                        axis=AX.X)
                nc.scalar.activation(n2[:], n2[:], ACT.Sqrt)
                nc.vector.reciprocal(n2[:], n2[:])
                ib_ps = psS.tile([128, 1], FP32, tag="small")
                nc.tensor.matmul(out=ib_ps[:], lhsT=ones_t[:], rhs=n2[:],
                                 start=True, stop=True)
                ib = wp.tile([128, 1], FP32, tag=f"invbs{lname}")
                nc.vector.tensor_copy(ib[:], ib_ps[:])
                return ib

            inv1b = inv_norm_b(p1f_t, "l1")
            inv2b = inv_norm_b(p2f_t, "l2")

            # ---------------- histogram k-th threshold ---------------------
            NF2 = NCORES * ZRO // 128
            S_big = wp.tile([128, NF2 * NBINS], BF16, tag="Sbig")

            def topk_tau(zt, nfree, k, lname):
                """zt: [128, nfree] fp32 scores (pads/masked = -BIG).
                returns [128,1] tile with the k-th-largest threshold."""
                mm = wp.tile([128, 2], FP32, tag="mm")
                msk = wp.tile([128, nfree], FP32, tag=f"hmsk{lname}")
                nc.vector.tensor_scalar(msk[:], zt[:], -1e29, 2e30, OP.is_lt,
                                        OP.mult)
                nc.vector.tensor_tensor(out=msk[:], in0=msk[:], in1=zt[:],
                                        op=OP.add)
                nc.vector.tensor_reduce(out=mm[:, 0:1], in_=msk[:], op=OP.min,
                                        axis=AX.X)
                nc.vector.tensor_reduce(out=mm[:, 1:2], in_=zt[:], op=OP.max,
                                        axis=AX.X)
                ztb = wp.tile([128, nfree], BF16, tag=f"ztb{lname}")
                nc.vector.tensor_copy(ztb[:], zt[:])
                lw = wp.tile([1, 2], FP32, tag="lw")  # [lo, w]
                mmT = wp.tile([1, 2, 128], FP32, tag="mmTs")
                for col in range(2):
                    mmT_ps = psS.tile([1, 128], FP32, tag="small")
                    nc.tensor.transpose(out=mmT_ps[:], in_=mm[:, col:col + 1],
                                        identity=idf_t[:])
                    nc.vector.tensor_copy(mmT[:, col, :], mmT_ps[:])
                nc.vector.tensor_reduce(out=lw[:, 0:1], in_=mmT[:, 0, :],
                                        op=OP.min, axis=AX.X)
                nc.vector.tensor_reduce(out=lw[:, 1:2], in_=mmT[:, 1, :],
                                        op=OP.max, axis=AX.X)
                nc.vector.tensor_scalar_add(lw[:, 0:1], lw[:, 0:1], -1e-3)
                nc.vector.tensor_scalar_add(lw[:, 1:2], lw[:, 1:2], 1e-3)
                nc.vector.tensor_tensor(out=lw[:, 1:2], in0=lw[:, 1:2],
                                        in1=lw[:, 0:1], op=OP.subtract)
                nc.vector.tensor_scalar_mul(lw[:, 1:2], lw[:, 1:2], 1.0 / NBINS)

                for st in range(NSTAGES):
                    lwb_ps = psS.tile([128, 2], FP32, tag="small")
                    nc.tensor.matmul(out=lwb_ps[:], lhsT=ones_t[:], rhs=lw[:],
                                     start=True, stop=True)
                    lwb = wp.tile([128, 2], FP32, tag="lwbs")
                    nc.vector.tensor_copy(lwb[:], lwb_ps[:])
                    tt = wp.tile([128, NBINS], FP32, tag="tt")
                    nc.vector.tensor_scalar(tt[:], iob_t[:], lwb[:, 1:2],
                                            lwb[:, 0:1], OP.mult, OP.add)
                    ttb = wp.tile([128, NBINS], BF16, tag="ttb")
                    nc.vector.tensor_copy(ttb[:], tt[:])
                    # S[p, j, n]: count-reduce over n is contiguous
                    S = S_big[:, :NBINS * nfree].rearrange(
                        "p (j n) -> p j n", j=NBINS)
                    nc.vector.tensor_tensor(
                        out=S,
                        in0=ztb[:].unsqueeze(1).broadcast_to(
                            [128, NBINS, nfree]),
                        in1=ttb[:].unsqueeze(2).broadcast_to(
                            [128, NBINS, nfree]),
                        op=OP.is_ge)
                    cntp = wp.tile([128, NBINS], BF16, tag="cntp")
                    with nc.allow_low_precision(
                            reason="counts <= nfree are exact in bf16"):
                        nc.vector.tensor_reduce(
                            out=cntp[:], in_=S, op=OP.add, axis=AX.X)
                    cnt_ps = psS.tile([1, NBINS], FP32, tag="small")
                    nc.tensor.matmul(out=cnt_ps[:], lhsT=onesPb_t[:],
                                     rhs=cntp[:], start=True, stop=True)
                    fl = wp.tile([1, NBINS], FP32, tag="fl")
                    nc.vector.tensor_scalar(fl[:], cnt_ps[:], float(k), None,
                                            OP.is_ge)
                    js = wp.tile([1, 1], FP32, tag="js")
                    nc.vector.tensor_reduce(out=js[:], in_=fl[:], op=OP.add,
                                            axis=AX.X)
                    nc.vector.tensor_scalar_add(js[:], js[:], -1.0)
                    nc.vector.tensor_scalar(lw[:, 0:1], js[:], lw[:, 1:2],
                                            lw[:, 0:1], OP.mult, OP.add)
                    if st != NSTAGES - 1:
                        nc.vector.tensor_scalar_mul(lw[:, 1:2], lw[:, 1:2],
                                                    1.0 / NBINS)
                taub_ps = psS.tile([128, 1], FP32, tag="small")
                nc.tensor.matmul(out=taub_ps[:], lhsT=ones_t[:],
                                 rhs=lw[:, 0:1], start=True, stop=True)
                taub = wp.tile([128, 1], FP32, tag=f"taubs{lname}")
                nc.vector.tensor_copy(taub[:], taub_ps[:])
                return taub

            # ======================= layer 1 ===============================
            h1 = bigp.tile([128, NCH * HPAD], BF16, tag="h1_all")
            z1 = wp.tile([128, NCH], FP32, tag="z1")
            # aggT tiles shared between layers (L1 uses the first two; its
            # dense reads complete before L2 overwrites them)
            aggT2 = [bigp.tile([128, NPAD], BF16, tag=f"aggT2_{fc}",
                               name=f"aggT2_{fc}")
                     for fc in range(4)]
            aggT1 = aggT2[:2]

            for b in range(NCH):
                agg_ps = psA.tile([128, HPAD], FP32, tag="aggps")
                for k in range(NB):
                    B = b * NB + k
                    xeb = sp.tile([128, 8, FIN], BF16, tag="xeb", bufs=2)
                    nc.sync.dma_start(
                        xeb[:].rearrange("p a f -> p (a f)"),
                        xe[:, B * 8 * FIN:(B + 1) * 8 * FIN])
                    wohb = sp.tile([128, 8, 128], BF16, tag="wohb1")
                    nc.sync.dma_start(
                        wohb[:].rearrange("p a d -> p (a d)"),
                        woh[:, B * 1024:(B + 1) * 1024])
                    for j in range(8):
                        nc.tensor.matmul(
                            out=agg_ps[:, :FIN], lhsT=wohb[:, j, :],
                            rhs=xeb[:, j, :],
                            start=(k == 0 and j == 0),
                            stop=(k == NB - 1 and j == 7))
                # transpose agg -> aggT1 chunks
                aggc = wp.tile([128, FIN], BF16, tag="aggc", bufs=2)
                nc.scalar.activation(aggc[:], agg_ps[:, :FIN], ACT.Copy)
                for fc in range(2):
                    tps = psT.tile([128, 128], BF16, tag="tps")
                    nc.tensor.transpose(out=tps[:],
                                        in_=aggc[:, fc * 128:(fc + 1) * 128],
                                        identity=idb_t[:])
                    nc.scalar.activation(aggT1[fc][:, b * 128:(b + 1) * 128],
                                         tps[:], ACT.Copy)
                # dense: h = relu(b1 + aggT.T @ w1relT + xT.T @ w1rootT)
                hp = psB.tile([128, HPAD], FP32, tag="hps")
                nc.tensor.matmul(out=hp[:], lhsT=ones1b_t[:], rhs=b1row_t[:],
                                 start=True, stop=False)
                for fc in range(2):
                    nc.tensor.matmul(
                        out=hp[:], lhsT=aggT1[fc][:, b * 128:(b + 1) * 128],
                        rhs=w1rel_t[fc][:], start=False, stop=False)
                for fc in range(2):
                    nc.tensor.matmul(
                        out=hp[:], lhsT=xT_t[fc][:, b * 128:(b + 1) * 128],
                        rhs=w1root_t[fc][:], start=False, stop=(fc == 1))
                hc = h1[:, b * HPAD:(b + 1) * HPAD]
                nc.scalar.activation(hc, hp[:], ACT.Relu)
                # z score (fp32)
                scr = wp.tile([128, HPAD], FP32, tag="scr", bufs=2)
                nc.vector.tensor_tensor(out=scr[:], in0=hc, in1=p1r_t[:],
                                        op=OP.mult)
                nc.vector.tensor_reduce(out=z1[:, b:b + 1], in_=scr[:],
                                        op=OP.add, axis=AX.X)
                # table row: [h | z | pad]
                tblb = wp.tile([128, ROWB], BF16, tag="tblb", bufs=2)
                nc.scalar.activation(tblb[:, 0:HPAD], hp[:], ACT.Relu)
                nc.vector.tensor_copy(
                    tblb[:, HPAD:HPAD + 2].bitcast(FP32), z1[:, b:b + 1])
                nc.sync.dma_start(tbl[b * 128:(b + 1) * 128, :], tblb[:])

            # masked z for selection
            pm30 = wp.tile([128, NCH], FP32, tag="pm30")
            nc.vector.tensor_scalar(pm30[:], pad_t[:], 1.0, BIG, OP.subtract,
                                    OP.mult)
            zm1 = wp.tile([128, NCH], FP32, tag="zm1")
            nc.vector.tensor_tensor(out=zm1[:], in0=z1[:], in1=pad_t[:],
                                    op=OP.mult)
            nc.vector.tensor_tensor(out=zm1[:], in0=zm1[:], in1=pm30[:],
                                    op=OP.add)
            nc.sync.dma_start(
                zsh1[:].rearrange("(b p) o -> p (b o)", p=128), zm1[:])
            nc.gpsimd.collective_compute(
                "AllGather", OP.bypass, replica_groups=RG,
                ins=[tbl[:]], outs=[tblag[:]])
            nc.gpsimd.collective_compute(
                "AllGather", OP.bypass, replica_groups=RG,
                ins=[zsh1[:]], outs=[zag1[:]])

            zt1 = wp.tile([128, NROWS // 128], FP32, tag="zt1")
            nc.sync.dma_start(
                zt1[:], zag1[:].rearrange("(p f) o -> p (f o)", p=128))
            tau1b = topk_tau(zt1, NROWS // 128, K1, "l1")

            # a1 per local bin + kept masks
            kp1 = wp.tile([128, NCH], FP32, tag="kp1")
            nc.vector.tensor_scalar(kp1[:], zm1[:], tau1b[:, 0:1], None,
                                    OP.is_ge)
            s1 = wp.tile([128, NCH], FP32, tag="s1")
            nc.scalar.activation(s1[:], z1[:], ACT.Tanh, scale=inv1b[:, 0:1])
            a1 = wp.tile([128, NCH], FP32, tag="a1")
            nc.vector.tensor_tensor(out=a1[:], in0=s1[:], in1=kp1[:],
                                    op=OP.mult)
            km30 = wp.tile([128, NCH], FP32, tag="km30")
            nc.vector.tensor_scalar(km30[:], kp1[:], 1.0, BIG, OP.subtract,
                                    OP.mult)

            # g1 (scaled, masked transpose) + readout 1
            gmT1 = [bigp.tile([128, NPAD], BF16, tag=f"gmT1_{fc}",
                              name=f"gmT1_{fc}")
                    for fc in range(4)]
            ro1s_ps = psS.tile([1, HPAD], FP32, tag="rosum")
            for b in range(NCH):
                hc = h1[:, b * HPAD:(b + 1) * HPAD]
                g1c = wp.tile([128, HPAD], BF16, tag="g1c", bufs=2)
                nc.vector.tensor_scalar(g1c[:], hc, a1[:, b:b + 1], None,
                                        OP.mult)
                nc.tensor.matmul(out=ro1s_ps[:], lhsT=onesPb_t[:], rhs=g1c[:],
                                 start=(b == 0), stop=(b == NCH - 1))
                gmc = wp.tile([128, HPAD], BF16, tag="gmc", bufs=2)
                nc.vector.tensor_scalar(gmc[:], hc, a1[:, b:b + 1],
                                        km30[:, b:b + 1], OP.mult, OP.add)
                for fc in range(4):
                    tps = psT.tile([128, 128], BF16, tag="tps")
                    nc.tensor.transpose(out=tps[:],
                                        in_=gmc[:, fc * 128:(fc + 1) * 128],
                                        identity=idb_t[:])
                    nc.scalar.activation(gmT1[fc][:, b * 128:(b + 1) * 128],
                                         tps[:], ACT.Copy)
            m1T = wp.tile([128, 4], FP32, tag="m1T")
            for fc in range(4):
                nc.vector.tensor_reduce(out=m1T[:, fc:fc + 1], in_=gmT1[fc][:],
                                        op=OP.max, axis=AX.X)
            ro1s = wp.tile([1, HPAD], FP32, tag="ro1s")
            nc.vector.tensor_copy(ro1s[:], ro1s_ps[:])
            # ro1 rides in the z2ro payload (rows NPAD.. and NPAD+512..)
            nc.sync.dma_start(z2ro[NPAD:NPAD + HPAD, :]
                              .rearrange("f o -> o f"), ro1s[:])
            nc.sync.dma_start(
                z2ro[NPAD + HPAD:NPAD + 1024, :]
                .rearrange("(c p) o -> p (c o)", p=128), m1T[:])

            # ======================= layer 2 ===============================
            h2 = bigp.tile([128, NCH * HPAD], BF16, tag="h2_all")
            z2 = wp.tile([128, NCH], FP32, tag="z2")

            for b in range(NCH):
                agg_ps = psA.tile([128, HPAD], FP32, tag="aggps")
                for k in range(NB):
                    B = b * NB + k
                    gt = gp.tile([128, 8, ROWB], BF16, tag="gath", bufs=3)
                    nc.gpsimd.dma_gather(
                        gt[:], tblag[:], idx2_t[:, B * 64:(B + 1) * 64],
                        1024, 1024, ROWB)
                    wohb = sp.tile([128, 8, 128], BF16, tag="wohb2", bufs=6)
                    nc.sync.dma_start(
                        wohb[:].rearrange("p a d -> p (a d)"),
                        woh[:, B * 1024:(B + 1) * 1024])
                    # per-slot scale a1 = tanh(z*inv)*(z>=tau)
                    zg = gt[:, :, HPAD:HPAD + 2].bitcast(FP32) \
                        .rearrange("p a o -> p (a o)")
                    kp8 = wp.tile([128, 8], FP32, tag="kp8", bufs=2)
                    nc.vector.tensor_scalar(kp8[:], zg, tau1b[:, 0:1], None,
                                            OP.is_ge)
                    s8 = wp.tile([128, 8], FP32, tag="s8", bufs=2)
                    nc.scalar.activation(s8[:], zg, ACT.Tanh,
                                         scale=inv1b[:, 0:1])
                    a1s = wp.tile([128, 8], BF16, tag="a1s", bufs=2)
                    nc.vector.tensor_tensor(out=a1s[:], in0=s8[:], in1=kp8[:],
                                            op=OP.mult)
                    ohs = wp.tile([128, 8, 128], BF16, tag="ohs", bufs=2)
                    nc.vector.tensor_tensor(
                        out=ohs[:], in0=wohb[:],
                        in1=a1s[:].unsqueeze(2).broadcast_to([128, 8, 128]),
                        op=OP.mult)
                    for j in range(8):
                        nc.tensor.matmul(
                            out=agg_ps[:], lhsT=ohs[:, j, :],
                            rhs=gt[:, j, 0:HPAD],
                            start=(k == 0 and j == 0),
                            stop=(k == NB - 1 and j == 7))
                aggc = wp.tile([128, HPAD], BF16, tag="aggc2", bufs=2)
                nc.scalar.activation(aggc[:], agg_ps[:], ACT.Copy)
                for fc in range(4):
                    tps = psT.tile([128, 128], BF16, tag="tps")
                    nc.tensor.transpose(out=tps[:],
                                        in_=aggc[:, fc * 128:(fc + 1) * 128],
                                        identity=idb_t[:])
                    nc.scalar.activation(aggT2[fc][:, b * 128:(b + 1) * 128],
                                         tps[:], ACT.Copy)
                hp = psB.tile([128, HPAD], FP32, tag="hps")
                nc.tensor.matmul(out=hp[:], lhsT=ones1b_t[:], rhs=b2row_t[:],
                                 start=True, stop=False)
                for fc in range(4):
                    nc.tensor.matmul(
                        out=hp[:], lhsT=aggT2[fc][:, b * 128:(b + 1) * 128],
                        rhs=w2rel_t[fc][:], start=False, stop=False)
                for fc in range(4):
                    nc.tensor.matmul(
                        out=hp[:], lhsT=gmT1[fc][:, b * 128:(b + 1) * 128],
                        rhs=w2root_t[fc][:], start=False, stop=(fc == 3))
                hc = h2[:, b * HPAD:(b + 1) * HPAD]
                nc.scalar.activation(hc, hp[:], ACT.Relu)
                scr = wp.tile([128, HPAD], FP32, tag="scr", bufs=2)
                nc.vector.tensor_tensor(out=scr[:], in0=hc, in1=p2r_t[:],
                                        op=OP.mult)
                nc.vector.tensor_reduce(out=z2[:, b:b + 1], in_=scr[:],
                                        op=OP.add, axis=AX.X)

            # masked z2 (kept-in-l1 only) -> z2ro payload -> AllGather
            zm2 = wp.tile([128, NCH], FP32, tag="zm2")
            nc.vector.tensor_tensor(out=zm2[:], in0=z2[:], in1=kp1[:],
                                    op=OP.mult)
            nc.vector.tensor_tensor(out=zm2[:], in0=zm2[:], in1=km30[:],
                                    op=OP.add)
            nc.sync.dma_start(
                z2ro[0:NPAD, :].rearrange("(b p) o -> p (b o)", p=128),
                zm2[:])
            nc.gpsimd.collective_compute(
                "AllGather", OP.bypass, replica_groups=RG,
                ins=[z2ro[:]], outs=[z2roag[:]])

            # tau2 over the masked flat payload
            ztr = wp.tile([128, NF2], FP32, tag="ztr")
            nc.sync.dma_start(
                ztr[:], z2roag[:].rearrange("(p f) o -> p (f o)", p=128))
            zt2 = wp.tile([128, NF2], FP32, tag="zt2")
            nc.vector.tensor_tensor(out=zt2[:], in0=ztr[:], in1=rom_t[:],
                                    op=OP.mult)
            rm30 = wp.tile([128, NF2], FP32, tag="rm30")
            nc.vector.tensor_scalar(rm30[:], rom_t[:], 1.0, BIG, OP.subtract,
                                    OP.mult)
            nc.vector.tensor_tensor(out=zt2[:], in0=zt2[:], in1=rm30[:],
                                    op=OP.add)
            tau2b = topk_tau(zt2, NF2, K2, "l2")

            kp2 = wp.tile([128, NCH], FP32, tag="kp2")
            nc.vector.tensor_scalar(kp2[:], zm2[:], tau2b[:, 0:1], None,
                                    OP.is_ge)
            s2 = wp.tile([128, NCH], FP32, tag="s2")
            nc.scalar.activation(s2[:], z2[:], ACT.Tanh, scale=inv2b[:, 0:1])
            a2 = wp.tile([128, NCH], FP32, tag="a2")
            nc.vector.tensor_tensor(out=a2[:], in0=s2[:], in1=kp2[:],
                                    op=OP.mult)
            km30b = wp.tile([128, NCH], FP32, tag="km30b")
            nc.vector.tensor_scalar(km30b[:], kp2[:], 1.0, BIG, OP.subtract,
                                    OP.mult)

            ro2s_ps = psS.tile([1, HPAD], FP32, tag="rosum")
            m2T = wp.tile([128, 4], FP32, tag="m2T")
            nc.vector.memset(m2T[:], -1e30)
            for b in range(NCH):
                hc = h2[:, b * HPAD:(b + 1) * HPAD]
                g2c = wp.tile([128, HPAD], BF16, tag="g1c", bufs=2)
                nc.vector.tensor_scalar(g2c[:], hc, a2[:, b:b + 1], None,
                                        OP.mult)
                nc.tensor.matmul(out=ro2s_ps[:], lhsT=onesPb_t[:], rhs=g2c[:],
                                 start=(b == 0), stop=(b == NCH - 1))
                gmc = wp.tile([128, HPAD], BF16, tag="gmc", bufs=2)
                nc.vector.tensor_scalar(gmc[:], hc, a2[:, b:b + 1],
                                        km30b[:, b:b + 1], OP.mult, OP.add)
                for fc in range(4):
                    tps = psT.tile([128, 128], BF16, tag="tps")
                    nc.tensor.transpose(out=tps[:],
                                        in_=gmc[:, fc * 128:(fc + 1) * 128],
                                        identity=idb_t[:])
                    red = wp.tile([128, 1], FP32, tag="redm", bufs=2)
                    nc.vector.tensor_reduce(out=red[:], in_=tps[:],
                                            op=OP.max, axis=AX.X)
                    nc.vector.tensor_tensor(out=m2T[:, fc:fc + 1],
                                            in0=m2T[:, fc:fc + 1],
                                            in1=red[:], op=OP.max)
            ro2s = wp.tile([1, HPAD], FP32, tag="ro2s")
            nc.vector.tensor_copy(ro2s[:], ro2s_ps[:])
            nc.sync.dma_start(ro2in[0:1, :], ro2s[:])
            nc.sync.dma_start(
                ro2in[1:2, :].rearrange("o (c p) -> p (o c)", p=128), m2T[:])
            nc.gpsimd.collective_compute(
                "AllGather", OP.bypass, replica_groups=RG,
                ins=[ro2in[:]], outs=[ro2ag[:]])

            # ======================= readout combine + head ================
            # ro1 lives in z2roag rows [s*ZRO+NPAD, s*ZRO+NPAD+1024)
            mx1 = wp.tile([128, 4], FP32, tag="mx1")
            mn1 = wp.tile([128, 4], FP32, tag="mn1")
            sums1 = wp.tile([128, 4, NCORES], FP32, tag="cmb1")
            maxs1 = wp.tile([128, 4, NCORES], FP32, tag="cmbm1")
            for s in range(NCORES):
                base = s * ZRO + NPAD
                nc.sync.dma_start(
                    sums1[:, :, s],
                    z2roag[base:base + HPAD, :]
                    .rearrange("(c p) o -> p (c o)", p=128))
                nc.sync.dma_start(
                    maxs1[:, :, s],
                    z2roag[base + HPAD:base + 1024, :]
                    .rearrange("(c p) o -> p (c o)", p=128))
            nc.vector.tensor_reduce(out=mn1[:], in_=sums1[:], op=OP.add,
                                    axis=AX.X)
            nc.vector.tensor_reduce(out=mx1[:], in_=maxs1[:], op=OP.max,
                                    axis=AX.X)
            nc.vector.tensor_scalar_mul(mn1[:], mn1[:], 1.0 / K1)

            mx2 = wp.tile([128, 4], FP32, tag="mx2")
            mn2 = wp.tile([128, 4], FP32, tag="mn2")
            sums2 = wp.tile([128, 4, 2 * NCORES], FP32, tag="cmb2")
            for r in range(2 * NCORES):
                nc.sync.dma_start(
                    sums2[:, :, r],
                    ro2ag[r:r + 1, :].rearrange("o (c p) -> p (o c)", p=128))
            s_ap = sums2[:].rearrange("p c (s t) -> p c t s", t=2)
            nc.vector.tensor_reduce(out=mn2[:], in_=s_ap[:, :, 0, :],
                                    op=OP.add, axis=AX.X)
            nc.vector.tensor_reduce(out=mx2[:], in_=s_ap[:, :, 1, :],
                                    op=OP.max, axis=AX.X)
            nc.vector.tensor_scalar_mul(mn2[:], mn2[:], 1.0 / K2)

            zT = wp.tile([128, 8], FP32, tag="zT")
            nc.vector.tensor_tensor(out=zT[:, 0:4], in0=mx1[:], in1=mx2[:],
                                    op=OP.add)
            nc.vector.tensor_tensor(out=zT[:, 4:8], in0=mn1[:], in1=mn2[:],
                                    op=OP.add)
            zTb = wp.tile([128, 8], BF16, tag="zTb")
            nc.vector.tensor_copy(zTb[:], zT[:])

            # lin1 replicated: z1h [1, 2048] via psum-row matmuls
            b1h_t = load(b1h, BF16)
            qt = [psA.tile([128, HPAD], FP32, tag="aggps", name=f"hq{q}")
                  if q < 2 else
                  psB.tile([128, HPAD], FP32, tag="hps", name=f"hq{q}")
                  for q in range(4)]
            for q in range(4):
                nc.tensor.matmul(out=qt[q][0:1, :], lhsT=ones11_t[:],
                                 rhs=b1h_t[:, q * 512:(q + 1) * 512],
                                 start=True, stop=False, skip_group_check=True)
            for t in range(8):
                l1c = sp.tile([128, 2048], BF16, tag="l1s", bufs=2)
                nc.sync.dma_start(l1c[:], l1T[t * 128:(t + 1) * 128, :])
                for q in range(4):
                    nc.tensor.matmul(
                        out=qt[q][0:1, :], lhsT=zTb[:, t:t + 1],
                        rhs=l1c[:, q * 512:(q + 1) * 512],
                        start=False, stop=(t == 7), skip_group_check=True)
            z1h = wp.tile([1, 2048], BF16, tag="z1h")
            for q in range(4):
                nc.scalar.activation(z1h[:, q * 512:(q + 1) * 512],
                                     qt[q][0:1, :], ACT.Relu)
            z1hT = wp.tile([128, 16], BF16, tag="z1hT")
            for t in range(16):
                tpv = psT.tile([128, 1], BF16, tag="tps")
                nc.tensor.transpose(out=tpv[:],
                                    in_=z1h[:, t * 128:(t + 1) * 128],
                                    identity=ones11_t[:])
                nc.scalar.activation(z1hT[:, t:t + 1], tpv[:], ACT.Copy)

            # lin2 shard (500 rows), same psum-row form
            b2h_t = load(b2h, BF16)
            o2p = psA.tile([128, HPAD], FP32, tag="aggps")
            nc.tensor.matmul(out=o2p[0:1, 0:512], lhsT=ones11_t[:],
                             rhs=b2h_t[:], start=True, stop=False,
                             skip_group_check=True)
            for t in range(16):
                l2c = sp.tile([128, 500], BF16, tag="l2s")
                nc.sync.dma_start(l2c[:], l2T[t * 128:(t + 1) * 128, :])
                nc.tensor.matmul(out=o2p[0:1, 0:500], lhsT=z1hT[:, t:t + 1],
                                 rhs=l2c[:], start=False, stop=(t == 15),
                                 skip_group_check=True)
            z2h = wp.tile([1, HPAD], BF16, tag="z2h")
            nc.vector.memset(z2h[:], 0.0)
            nc.scalar.activation(z2h[:, 0:500], o2p[0:1, 0:500], ACT.Relu)
            z2hT = wp.tile([128, 4], BF16, tag="z2hT")
            for t in range(4):
                tpv = psT.tile([128, 1], BF16, tag="tps")
                nc.tensor.transpose(out=tpv[:],
                                    in_=z2h[:, t * 128:(t + 1) * 128],
                                    identity=ones11_t[:])
                nc.scalar.activation(z2hT[:, t:t + 1], tpv[:], ACT.Copy)

            # lin3 partial (own contraction shard) + AllReduce
            l3_t = load_chunks(l3T, 4, 128, "l3Tc")
            b3h_t = load(b3h)
            o3p = psB.tile([128, 1], FP32, tag="hps")
            for t in range(4):
                nc.tensor.matmul(out=o3p[:], lhsT=l3_t[t][:],
                                 rhs=z2hT[:, t:t + 1],
                                 start=(t == 0), stop=(t == 3))
            o3 = wp.tile([128, 1], FP32, tag="o3")
            nc.vector.tensor_copy(o3[:], o3p[:])
            nc.sync.dma_start(oin[:], o3[:])
            nc.gpsimd.collective_compute(
                "AllReduce", OP.add, replica_groups=RG,
                ins=[oin[:]], outs=[oar[:]])
            fin = wp.tile([128, 1], FP32, tag="fin")
            nc.sync.dma_start(fin[:], oar[:])
            nc.scalar.activation(fin[:], fin[:], ACT.Sigmoid,
                                 bias=b3h_t[:, 0:1])
            nc.sync.dma_start(out[:].rearrange("o f -> f o"), fin[:NOUT, :])

    nc.compile()
    return nc


# ---------------------------------------------------------------------------
# entry point
# ---------------------------------------------------------------------------

_CACHE = {}
TRACE = False


def kernel(**inputs):
    prep = _pack(inputs["x"], inputs["edge_src"], inputs["edge_dst"],
                 inputs["edge_weight"])
    if "nc" not in _CACHE:
        _CACHE["nc"] = _build()
    nc = _CACHE["nc"]
    in_maps = _host_inputs(inputs, prep)
    res = bass_utils.run_bass_kernel_spmd(
        nc, in_maps, core_ids=list(range(NCORES)), trace=TRACE)
    kernel.last_results = res
    return res.results[0]["out"]


if __name__ == "__main__":
    dat = np.load("/tmp/inputs.npz")
    inputs = {k: dat[k] for k in dat.files}
    got = kernel(**inputs)
    exp = np.load("/tmp/expected.npy")
    err = np.abs(got - exp).max()
    rel = err / np.abs(exp).max()
    print("out[0,:6] =", got[0, :6])
    print("exp[0,:6] =", exp[0, :6])
    print("max abs err:", err, "rel:", rel)


# revision 36
# speedup vs baseline: 1.0935x; 1.0033x over previous
"""Trainium2 Bass kernel for nn_Net_48301202211072 (GNN message passing).

2-layer GraphConv + TopKPooling + readout + MLP head, sharded over 8
NeuronCores. Strategy (v2):

- Nodes (and incident edges, grouped by destination) are sharded across
  cores. Edges are packed column-major into 11 destination bins per core
  (<=128 nodes and <=2048 edges per bin), 2 blocks of 1024 edge slots
  per bin. Everything is bf16 except score/threshold arithmetic.
- Layer-1 aggregation: the host pre-arranges per-edge source rows (xe)
  and edge-weighted one-hot matrices (Woh); the device streams both and
  aggregates with plain PE matmuls (no dma_gather, no per-edge DVE).
- Layer-2 aggregation: each node's table row [h1 | z1] is AllGather'd
  right after conv1 (before topk), so the big collective and the tau1
  histogram overlap; dma_gather fetches rows per edge and the per-slot
  scale tanh(z/||p||)*(z>=tau)*w is applied via the one-hot weights.
- TopK threshold: replicated 4-stage 64-bin histogram over the
  AllGather'd scores (bf16 compares, exact-enough within tolerance).
- Readout: ones-matmul mean + masked-transpose max, combined via small
  AllGathers (ro1 rides in the z2 AllGather payload).
- Head: lin1 replicated, lin2 sharded by rows, lin3 by contraction,
  one final AllReduce.
"""
import math
import sys

import numpy as np
import ml_dtypes

sys.path.insert(0, "/opt/trn_rl_repo")

import concourse.bacc as bacc  # noqa: E402
import concourse.mybir as mybir  # noqa: E402
import concourse.tile as tile  # noqa: E402
from concourse import bass_utils  # noqa: E402

FP32 = mybir.dt.float32
BF16 = mybir.dt.bfloat16
I16 = mybir.dt.int16
AX = mybir.AxisListType
OP = mybir.AluOpType
ACT = mybir.ActivationFunctionType
BFNP = ml_dtypes.bfloat16

NCORES = 8
N = 10000
FIN = 256
HID = 500
HPAD = 512
NOUT = 100
NPC = N // NCORES          # 1250 nodes per core
NCH = 11                   # dst bins per core (<=128 nodes, <=2048 edges)
NB = 2                     # blocks of 1024 edge slots per bin
BTOT = NCH * NB            # 22
NPAD = NCH * 128           # 1408 table rows per core
NROWS = NCORES * NPAD      # 11264
ROWB = 640                 # table row: 512 h bf16 + 2 z-as-bf16 + 126 pad
NBINS = 32
NSTAGES = 3
K1 = N // 2
K2 = N // 4
ZRO = NPAD + 1024          # 2432: zm2 + ro1 payload rows per core
SPLITB = 6                 # bins 0..5 AllGather'd early (rows 0:768)
SPLITR = SPLITB * 128      # 768
ASZ = NCORES * SPLITR      # 6144 rows in part A of the gathered table
BIG = 1e30


# ---------------------------------------------------------------------------
# host preprocessing
# ---------------------------------------------------------------------------

def _pack(x, edge_src, edge_dst, edge_weight):
    src = np.asarray(edge_src, np.int64)
    dst = np.asarray(edge_dst, np.int64)
    w = np.asarray(edge_weight, np.float32)
    x_bf = np.ascontiguousarray(np.asarray(x, np.float32)).astype(BFNP)

    # pass 1: per-core greedy bin boundaries + node->table-row map
    binrow = np.zeros((NCORES, NPC), np.int64)   # local node -> row in [0,NPAD)
    starts_all, counts_all = [], []
    for c in range(NCORES):
        lo = c * NPC
        m = (dst >= lo) & (dst < lo + NPC)
        ed = dst[m] - lo
        deg = np.bincount(ed, minlength=NPC)
        bstart, bnodes = [], []
        n0 = 0
        while n0 < NPC:
            e_acc, nn = 0, 0
            while n0 + nn < NPC and nn < 128 and e_acc + deg[n0 + nn] <= 2048:
                e_acc += deg[n0 + nn]
                nn += 1
            assert nn > 0
            bstart.append(n0)
            bnodes.append(nn)
            n0 += nn
        assert len(bstart) <= NCH, f"core {c} needs {len(bstart)} bins"
        while len(bstart) < NCH:
            bstart.append(NPC)
            bnodes.append(0)
        bstart = np.asarray(bstart, np.int64)
        bnodes = np.asarray(bnodes, np.int64)
        for b in range(NCH):
            s, nn = bstart[b], bnodes[b]
            binrow[c, s:s + nn] = b * 128 + np.arange(nn)
        starts_all.append(bstart)
        counts_all.append(bnodes)

    per_core = []
    for c in range(NCORES):
        lo = c * NPC
        m = (dst >= lo) & (dst < lo + NPC)
        es, ed, ew = src[m], dst[m] - lo, w[m]
        order = np.argsort(ed, kind="stable")
        es, ed, ew = es[order], ed[order], ew[order]
        bstart, bnodes = starts_all[c], counts_all[c]
        # edge ranges per bin (bins are consecutive node ranges)
        bin_edge_start = np.searchsorted(ed, bstart)
        bin_edge_end = np.searchsorted(ed, bstart + bnodes)

        # slot assignment (column-major within each bin's 2 blocks)
        srcslot = np.full(BTOT * 1024, -1, np.int64)
        dslot = np.zeros(BTOT * 1024, np.int64)
        wslot = np.zeros(BTOT * 1024, np.float32)
        for b in range(NCH):
            e0, e1 = bin_edge_start[b], bin_edge_end[b]
            cnt = e1 - e0
            assert cnt <= NB * 1024
            u = np.arange(cnt)
            blk = b * NB + u // 1024
            u2 = u % 1024
            pos = blk * 1024 + u2
            srcslot[pos] = es[e0:e1]
            dslot[pos] = ed[e0:e1] - bstart[b]
            wslot[pos] = ew[e0:e1]

        real = srcslot >= 0
        # xe: [128, BTOT*8*256] pre-gathered source rows, bf16
        rows = np.zeros((BTOT * 1024, FIN), BFNP)
        rows[real] = x_bf[srcslot[real]]
        xe = np.ascontiguousarray(
            rows.reshape(BTOT, 8, 128, FIN).transpose(2, 0, 1, 3)
            .reshape(128, BTOT * 8 * FIN))

        # woh: [128, BTOT*8*128] edge-weighted one-hots, bf16
        woh = np.zeros((128, BTOT * 8 * 128), np.float32)
        pos = np.nonzero(real)[0]
        blk = pos // 1024
        u2 = pos % 1024
        p = u2 % 128
        j = u2 // 128
        woh[p, (blk * 8 + j) * 128 + dslot[pos]] = wslot[pos]
        woh = woh.astype(BFNP)

        # idx2: slot -> row in the AllGather'd table
        sc = srcslot // NPC
        slo = srcslot - sc * NPC
        idx2 = np.zeros(BTOT * 1024, np.int64)
        idx2[real] = sc[real] * NPAD + binrow[sc[real], slo[real]]
        per_core.append(dict(xe=xe, woh=woh, idx2=idx2,
                             bstart=bstart, bnodes=bnodes))
    return per_core


def _wrap16(idx_flat):
    """[BTOT*1024] -> [128, BTOT*64] int16, per-block wrapped-16 replicated."""
    out = np.zeros((128, BTOT * 64), np.int16)
    for blk in range(BTOT):
        b = idx_flat[blk * 1024:(blk + 1) * 1024].astype(np.int16)
        t = b.reshape(64, 16).T          # [16, 64]
        out[:, blk * 64:(blk + 1) * 64] = np.tile(t, (8, 1))
    return out


def _host_inputs(inputs, prep):
    x = np.ascontiguousarray(np.asarray(inputs["x"], np.float32))

    def padT(a, rows, cols):
        out = np.zeros((rows, cols), np.float32)
        t = np.asarray(a, np.float32).T
        out[: t.shape[0], : t.shape[1]] = t
        return out.astype(BFNP)

    w1relT = padT(inputs["W1_rel"], FIN, HPAD)
    w1rootT = padT(inputs["W1_root"], FIN, HPAD)
    w2relT = padT(inputs["W2_rel"], HPAD, HPAD)
    w2rootT = padT(inputs["W2_root"], HPAD, HPAD)

    def rowv(v):
        out = np.zeros((1, HPAD), np.float32)
        vv = np.asarray(v, np.float32)
        out[0, : vv.shape[0]] = vv
        return out

    b1row = rowv(inputs["b1"]).astype(BFNP)
    b2row = rowv(inputs["b2"]).astype(BFNP)
    p1f = rowv(inputs["p1_w"])
    p2f = rowv(inputs["p2_w"])
    p1r = np.tile(p1f, (128, 1)).astype(BFNP)
    p2r = np.tile(p2f, (128, 1)).astype(BFNP)

    iotaB = np.tile(np.arange(NBINS, dtype=np.float32)[None, :], (128, 1))
    identb = np.eye(128, dtype=np.float32).astype(BFNP)
    identf = np.eye(128, dtype=np.float32)
    ones128f = np.ones((1, 128), np.float32)
    ones1b = np.ones((1, 128), np.float32).astype(BFNP)
    onesPb = np.ones((128, 1), np.float32).astype(BFNP)
    ones11 = np.ones((1, 1), np.float32).astype(BFNP)

    # mask of z positions inside the flattened z2ro AllGather payload
    g = np.arange(128 * (NCORES * ZRO // 128), dtype=np.int64)
    romask = ((g % ZRO) < NPAD).astype(np.float32).reshape(
        128, NCORES * ZRO // 128)

    lin1W = np.asarray(inputs["lin1_W"], np.float32)   # [2000, 1000]
    lin2W = np.asarray(inputs["lin2_W"], np.float32)   # [4000, 2000]
    lin3W = np.asarray(inputs["lin3_W"], np.float32)   # [100, 4000]
    lin1b = np.asarray(inputs["lin1_b"], np.float32)
    lin2b = np.asarray(inputs["lin2_b"], np.float32)
    lin3b = np.asarray(inputs["lin3_b"], np.float32)

    # lin1 replicated: rows = z layout [max 0:500 | pad | mean 512:1012 | pad]
    l1T = np.zeros((1024, 2048), np.float32)
    sh = lin1W.T                                       # [1000, 2000]
    l1T[:500, :2000] = sh[:500]
    l1T[512:1012, :2000] = sh[500:]
    l1T = l1T.astype(BFNP)
    b1h = np.zeros((1, 2048), np.float32)
    b1h[0, :2000] = lin1b
    b1h = b1h.astype(BFNP)

    per_core = []
    for c in range(NCORES):
        pr = prep[c]
        bstart, bnodes = pr["bstart"], pr["bnodes"]

        xT = np.zeros((FIN, NPAD), np.float32)
        padmask = np.zeros((128, NCH), np.float32)
        for b in range(NCH):
            s, nn = bstart[b], bnodes[b]
            if nn:
                xT[:, b * 128: b * 128 + nn] = x[c * NPC + s: c * NPC + s + nn].T
                padmask[:nn, b] = 1.0
        xT = xT.astype(BFNP)

        l2T = np.zeros((2048, 500), np.float32)
        l2T[:2000] = lin2W[c * 500:(c + 1) * 500].T
        l2T = l2T.astype(BFNP)
        b2h = np.zeros((1, 512), np.float32)
        b2h[0, :500] = lin2b[c * 500:(c + 1) * 500]
        b2h = b2h.astype(BFNP)

        l3T = np.zeros((512, 128), np.float32)
        l3T[:500, :NOUT] = lin3W[:, c * 500:(c + 1) * 500].T
        l3T = l3T.astype(BFNP)
        b3h = np.zeros((128, 1), np.float32)
        b3h[:NOUT, 0] = lin3b

        per_core.append(dict(
            xe=pr["xe"], woh=pr["woh"], idx2=_wrap16(pr["idx2"]),
            padmask=padmask, xT=xT,
            w1relT=w1relT, w1rootT=w1rootT, w2relT=w2relT, w2rootT=w2rootT,
            b1row=b1row, b2row=b2row, p1f=p1f, p2f=p2f, p1r=p1r, p2r=p2r,
            iotaB=iotaB, identb=identb, identf=identf, ones128f=ones128f,
            ones1b=ones1b, onesPb=onesPb, ones11=ones11,
            romask=romask,
            l1T=l1T, b1h=b1h, l2T=l2T, b2h=b2h, l3T=l3T, b3h=b3h,
        ))
    return per_core


# ---------------------------------------------------------------------------
# device program
# ---------------------------------------------------------------------------

def _build():
    nc = bacc.Bacc("TRN2", target_bir_lowering=False, debug=False,
                   num_devices=NCORES)

    def din(name, shape, dt=FP32):
        return nc.dram_tensor(name, shape, dt, kind="ExternalInput")

    xe = din("xe", [128, BTOT * 8 * FIN], BF16)
    woh = din("woh", [128, BTOT * 8 * 128], BF16)
    idx2 = din("idx2", [128, BTOT * 64], I16)
    padmask = din("padmask", [128, NCH])
    xT = din("xT", [FIN, NPAD], BF16)
    w1relT = din("w1relT", [FIN, HPAD], BF16)
    w1rootT = din("w1rootT", [FIN, HPAD], BF16)
    w2relT = din("w2relT", [HPAD, HPAD], BF16)
    w2rootT = din("w2rootT", [HPAD, HPAD], BF16)
    b1row = din("b1row", [1, HPAD], BF16)
    b2row = din("b2row", [1, HPAD], BF16)
    p1f = din("p1f", [1, HPAD])
    p2f = din("p2f", [1, HPAD])
    p1r = din("p1r", [128, HPAD], BF16)
    p2r = din("p2r", [128, HPAD], BF16)
    iotaB = din("iotaB", [128, NBINS])
    identb = din("identb", [128, 128], BF16)
    identf = din("identf", [128, 128])
    ones128f = din("ones128f", [1, 128])
    ones1b = din("ones1b", [1, 128], BF16)
    onesPb = din("onesPb", [128, 1], BF16)
    ones11 = din("ones11", [1, 1], BF16)
    romask = din("romask", [128, NCORES * ZRO // 128])
    l1T = din("l1T", [1024, 2048], BF16)
    b1h = din("b1h", [1, 2048], BF16)
    l2T = din("l2T", [2048, 500], BF16)
    b2h = din("b2h", [1, 512], BF16)
    l3T = din("l3T", [512, 128], BF16)
    b3h = din("b3h", [128, 1])

    out = nc.dram_tensor("out", [1, NOUT], FP32, kind="ExternalOutput")

    RG = [list(range(NCORES))]

    with tile.TileContext(nc) as tc:
        with (
            tc.tile_pool(name="const", bufs=1) as cp,
            tc.tile_pool(name="stream", bufs=3) as sp,
            tc.tile_pool(name="gather", bufs=2) as gp,
            tc.tile_pool(name="work", bufs=1) as wp,
            tc.tile_pool(name="big", bufs=1) as bigp,
            tc.tile_pool(name="psA", bufs=2, space="PSUM") as psA,
            tc.tile_pool(name="psB", bufs=2, space="PSUM") as psB,
            tc.tile_pool(name="psT", bufs=2, space="PSUM") as psT,
            tc.tile_pool(name="psS", bufs=1, space="PSUM") as psS,
            tc.tile_pool(name="dram", bufs=1, space="DRAM") as dr,
        ):
            def load(src, dt=FP32, tag=None):
                tl = cp.tile(list(src.shape), dt, tag=tag or src.name)
                nc.sync.dma_start(tl[:], src[:])
                return tl

            idx2_t = load(idx2, I16)
            pad_t = load(padmask)
            iob_t = load(iotaB)
            idb_t = load(identb, BF16)
            idf_t = load(identf)
            ones_t = load(ones128f)
            ones1b_t = load(ones1b, BF16)
            onesPb_t = load(onesPb, BF16)
            ones11_t = load(ones11, BF16)
            b1row_t = load(b1row, BF16)
            b2row_t = load(b2row, BF16)
            p1f_t = load(p1f)
            p2f_t = load(p2f)
            p1r_t = load(p1r, BF16)
            p2r_t = load(p2r, BF16)
            rom_t = load(romask)

            def load_chunks(src, nchunks, cols, tag, dt=BF16):
                ts = []
                for k in range(nchunks):
                    t = cp.tile([128, cols], dt, tag=f"{tag}{k}")
                    nc.sync.dma_start(t[:], src[k * 128:(k + 1) * 128, :cols])
                    ts.append(t)
                return ts

            w1rel_t = load_chunks(w1relT, 2, HPAD, "w1rel")
            w1root_t = load_chunks(w1rootT, 2, HPAD, "w1root")
            w2rel_t = load_chunks(w2relT, 4, HPAD, "w2rel")
            w2root_t = load_chunks(w2rootT, 4, HPAD, "w2root")
            xT_t = load_chunks(xT, 2, NPAD, "xTc")

            # DRAM internal tiles
            tbl = dr.tile([NPAD, ROWB], BF16)
            tblag = dr.tile([NROWS, ROWB], BF16, addr_space="Shared")
            zsh1 = dr.tile([NPAD, 1], FP32)
            zag1 = dr.tile([NROWS, 1], FP32, addr_space="Shared")
            z2ro = dr.tile([ZRO, 1], FP32)
            z2roag = dr.tile([NCORES * ZRO, 1], FP32, addr_space="Shared")
            ro2in = dr.tile([2, HPAD], FP32)
            ro2ag = dr.tile([2 * NCORES, HPAD], FP32, addr_space="Shared")
            oin = dr.tile([128, 1], FP32)
            oar = dr.tile([128, 1], FP32, addr_space="Shared")
            wrm = dr.tile([16, 1], FP32)
            wrmag = dr.tile([16 * NCORES, 1], FP32, addr_space="Shared")

            # collective-stack warmup: absorb first-collective setup cost
            # while layer 1 computes
            wz = wp.tile([16, 1], FP32, tag="wz")
            nc.vector.memset(wz[:], 0.0)
            nc.sync.dma_start(wrm[:], wz[:])
            nc.gpsimd.collective_compute(
                "AllGather", OP.bypass, replica_groups=RG,
                ins=[wrm[:]], outs=[wrmag[:]])

            # -------- norms first (Sqrt table load hides under L1) ---------
            def inv_norm_b(pf_t, lname):
                """[128,1] broadcast of 1/||p||."""
                sq = wp.tile([1, HPAD], FP32, tag="pnsq")
                nc.vector.tensor_tensor(out=sq[:], in0=pf_t[:], in1=pf_t[:],
                                        op=OP.mult)
                n2 = wp.tile([1, 1], FP32, tag="pn2")
                nc.vector.tensor_reduce(out=n2[:], in_=sq[:], op=OP.add,
                                        axis=AX.X)
                nc.scalar.activation(n2[:], n2[:], ACT.Sqrt)
                nc.vector.reciprocal(n2[:], n2[:])
                ib_ps = psS.tile([128, 1], FP32, tag="small")
                nc.tensor.matmul(out=ib_ps[:], lhsT=ones_t[:], rhs=n2[:],
                                 start=True, stop=True)
                ib = wp.tile([128, 1], FP32, tag=f"invbs{lname}")
                nc.vector.tensor_copy(ib[:], ib_ps[:])
                return ib

            inv1b = inv_norm_b(p1f_t, "l1")
            inv2b = inv_norm_b(p2f_t, "l2")

            # ---------------- histogram k-th threshold ---------------------
            NF2 = NCORES * ZRO // 128
            S_big = wp.tile([128, NF2 * NBINS], BF16, tag="Sbig")

            def topk_tau(zt, nfree, k, lname):
                """zt: [128, nfree] fp32 scores (pads/masked = -BIG).
                returns [128,1] tile with the k-th-largest threshold."""
                mm = wp.tile([128, 2], FP32, tag="mm")
                msk = wp.tile([128, nfree], FP32, tag=f"hmsk{lname}")
                nc.vector.tensor_scalar(msk[:], zt[:], -1e29, 2e30, OP.is_lt,
                                        OP.mult)
                nc.vector.tensor_tensor(out=msk[:], in0=msk[:], in1=zt[:],
                                        op=OP.add)
                nc.vector.tensor_reduce(out=mm[:, 0:1], in_=msk[:], op=OP.min,
                                        axis=AX.X)
                nc.vector.tensor_reduce(out=mm[:, 1:2], in_=zt[:], op=OP.max,
                                        axis=AX.X)
                ztb = wp.tile([128, nfree], BF16, tag=f"ztb{lname}")
                nc.vector.tensor_copy(ztb[:], zt[:])
                lw = wp.tile([1, 2], FP32, tag="lw")  # [lo, w]
                mmT = wp.tile([1, 2, 128], FP32, tag="mmTs")
                for col in range(2):
                    mmT_ps = psS.tile([1, 128], FP32, tag="small")
                    nc.tensor.transpose(out=mmT_ps[:], in_=mm[:, col:col + 1],
                                        identity=idf_t[:])
                    nc.vector.tensor_copy(mmT[:, col, :], mmT_ps[:])
                nc.vector.tensor_reduce(out=lw[:, 0:1], in_=mmT[:, 0, :],
                                        op=OP.min, axis=AX.X)
                nc.vector.tensor_reduce(out=lw[:, 1:2], in_=mmT[:, 1, :],
                                        op=OP.max, axis=AX.X)
                nc.vector.tensor_scalar_add(lw[:, 0:1], lw[:, 0:1], -1e-3)
                nc.vector.tensor_scalar_add(lw[:, 1:2], lw[:, 1:2], 1e-3)
                nc.vector.tensor_tensor(out=lw[:, 1:2], in0=lw[:, 1:2],
                                        in1=lw[:, 0:1], op=OP.subtract)
                nc.vector.tensor_scalar_mul(lw[:, 1:2], lw[:, 1:2], 1.0 / NBINS)

                for st in range(NSTAGES):
                    lwb_ps = psS.tile([128, 2], FP32, tag="small")
                    nc.tensor.matmul(out=lwb_ps[:], lhsT=ones_t[:], rhs=lw[:],
                                     start=True, stop=True)
                    lwb = wp.tile([128, 2], FP32, tag="lwbs")
                    nc.vector.tensor_copy(lwb[:], lwb_ps[:])
                    tt = wp.tile([128, NBINS], FP32, tag="tt")
                    nc.vector.tensor_scalar(tt[:], iob_t[:], lwb[:, 1:2],
                                            lwb[:, 0:1], OP.mult, OP.add)
                    ttb = wp.tile([128, NBINS], BF16, tag="ttb")
                    nc.vector.tensor_copy(ttb[:], tt[:])
                    # S[p, j, n]: count-reduce over n is contiguous
                    S = S_big[:, :NBINS * nfree].rearrange(
                        "p (j n) -> p j n", j=NBINS)
                    nc.vector.tensor_tensor(
                        out=S,
                        in0=ztb[:].unsqueeze(1).broadcast_to(
                            [128, NBINS, nfree]),
                        in1=ttb[:].unsqueeze(2).broadcast_to(
                            [128, NBINS, nfree]),
                        op=OP.is_ge)
                    cntp = wp.tile([128, NBINS], BF16, tag="cntp")
                    with nc.allow_low_precision(
                            reason="counts <= nfree are exact in bf16"):
                        nc.vector.tensor_reduce(
                            out=cntp[:], in_=S, op=OP.add, axis=AX.X)
                    cnt_ps = psS.tile([1, NBINS], FP32, tag="small")
                    nc.tensor.matmul(out=cnt_ps[:], lhsT=onesPb_t[:],
                                     rhs=cntp[:], start=True, stop=True)
                    fl = wp.tile([1, NBINS], FP32, tag="fl")
                    nc.vector.tensor_scalar(fl[:], cnt_ps[:], float(k), None,
                                            OP.is_ge)
                    js = wp.tile([1, 1], FP32, tag="js")
                    nc.vector.tensor_reduce(out=js[:], in_=fl[:], op=OP.add,
                                            axis=AX.X)
                    nc.vector.tensor_scalar_add(js[:], js[:], -1.0)
                    nc.vector.tensor_scalar(lw[:, 0:1], js[:], lw[:, 1:2],
                                            lw[:, 0:1], OP.mult, OP.add)
                    if st != NSTAGES - 1:
                        nc.vector.tensor_scalar_mul(lw[:, 1:2], lw[:, 1:2],
                                                    1.0 / NBINS)
                taub_ps = psS.tile([128, 1], FP32, tag="small")
                nc.tensor.matmul(out=taub_ps[:], lhsT=ones_t[:],
                                 rhs=lw[:, 0:1], start=True, stop=True)
                taub = wp.tile([128, 1], FP32, tag=f"taubs{lname}")
                nc.vector.tensor_copy(taub[:], taub_ps[:])
                return taub

            # ======================= layer 1 ===============================
            h1 = bigp.tile([128, NCH * HPAD], BF16, tag="h1_all")
            z1 = wp.tile([128, NCH], FP32, tag="z1")
            # aggT tiles shared between layers (L1 uses the first two; its
            # dense reads complete before L2 overwrites them)
            aggT2 = [bigp.tile([128, NPAD], BF16, tag=f"aggT2_{fc}",
                               name=f"aggT2_{fc}")
                     for fc in range(4)]
            aggT1 = aggT2[:2]

            for b in range(NCH):
                agg_ps = psA.tile([128, HPAD], FP32, tag="aggps")
                for k in range(NB):
                    B = b * NB + k
                    xeb = sp.tile([128, 8, FIN], BF16, tag="xeb", bufs=2)
                    nc.sync.dma_start(
                        xeb[:].rearrange("p a f -> p (a f)"),
                        xe[:, B * 8 * FIN:(B + 1) * 8 * FIN])
                    wohb = sp.tile([128, 8, 128], BF16, tag="wohb1")
                    nc.sync.dma_start(
                        wohb[:].rearrange("p a d -> p (a d)"),
                        woh[:, B * 1024:(B + 1) * 1024])
                    for j in range(8):
                        nc.tensor.matmul(
                            out=agg_ps[:, :FIN], lhsT=wohb[:, j, :],
                            rhs=xeb[:, j, :],
                            start=(k == 0 and j == 0),
                            stop=(k == NB - 1 and j == 7))
                # transpose agg -> aggT1 chunks
                aggc = wp.tile([128, FIN], BF16, tag="aggc", bufs=2)
                nc.scalar.activation(aggc[:], agg_ps[:, :FIN], ACT.Copy)
                for fc in range(2):
                    tps = psT.tile([128, 128], BF16, tag="tps")
                    nc.tensor.transpose(out=tps[:],
                                        in_=aggc[:, fc * 128:(fc + 1) * 128],
                                        identity=idb_t[:])
                    nc.scalar.activation(aggT1[fc][:, b * 128:(b + 1) * 128],
                                         tps[:], ACT.Copy)
                # dense: h = relu(b1 + aggT.T @ w1relT + xT.T @ w1rootT)
                hp = psB.tile([128, HPAD], FP32, tag="hps")
                nc.tensor.matmul(out=hp[:], lhsT=ones1b_t[:], rhs=b1row_t[:],
                                 start=True, stop=False)
                for fc in range(2):
                    nc.tensor.matmul(
                        out=hp[:], lhsT=aggT1[fc][:, b * 128:(b + 1) * 128],
                        rhs=w1rel_t[fc][:], start=False, stop=False)
                for fc in range(2):
                    nc.tensor.matmul(
                        out=hp[:], lhsT=xT_t[fc][:, b * 128:(b + 1) * 128],
                        rhs=w1root_t[fc][:], start=False, stop=(fc == 1))
                hc = h1[:, b * HPAD:(b + 1) * HPAD]
                nc.scalar.activation(hc, hp[:], ACT.Relu)
                # z score (fp32)
                scr = wp.tile([128, HPAD], FP32, tag="scr", bufs=2)
                nc.vector.tensor_tensor(out=scr[:], in0=hc, in1=p1r_t[:],
                                        op=OP.mult)
                nc.vector.tensor_reduce(out=z1[:, b:b + 1], in_=scr[:],
                                        op=OP.add, axis=AX.X)
                # table row: [h | z | pad]
                tblb = wp.tile([128, ROWB], BF16, tag="tblb", bufs=2)
                nc.scalar.activation(tblb[:, 0:HPAD], hp[:], ACT.Relu)
                nc.vector.tensor_copy(
                    tblb[:, HPAD:HPAD + 2].bitcast(FP32), z1[:, b:b + 1])
                nc.sync.dma_start(tbl[b * 128:(b + 1) * 128, :], tblb[:])

            # masked z for selection
            pm30 = wp.tile([128, NCH], FP32, tag="pm30")
            nc.vector.tensor_scalar(pm30[:], pad_t[:], 1.0, BIG, OP.subtract,
                                    OP.mult)
            zm1 = wp.tile([128, NCH], FP32, tag="zm1")
            nc.vector.tensor_tensor(out=zm1[:], in0=z1[:], in1=pad_t[:],
                                    op=OP.mult)
            nc.vector.tensor_tensor(out=zm1[:], in0=zm1[:], in1=pm30[:],
                                    op=OP.add)
            nc.sync.dma_start(
                zsh1[:].rearrange("(b p) o -> p (b o)", p=128), zm1[:])
            nc.gpsimd.collective_compute(
                "AllGather", OP.bypass, replica_groups=RG,
                ins=[tbl[:]], outs=[tblag[:]])
            nc.gpsimd.collective_compute(
                "AllGather", OP.bypass, replica_groups=RG,
                ins=[zsh1[:]], outs=[zag1[:]])

            zt1 = wp.tile([128, NROWS // 128], FP32, tag="zt1")
            nc.sync.dma_start(
                zt1[:], zag1[:].rearrange("(p f) o -> p (f o)", p=128))
            tau1b = topk_tau(zt1, NROWS // 128, K1, "l1")

            # a1 per local bin + kept masks
            kp1 = wp.tile([128, NCH], FP32, tag="kp1")
            nc.vector.tensor_scalar(kp1[:], zm1[:], tau1b[:, 0:1], None,
                                    OP.is_ge)
            s1 = wp.tile([128, NCH], FP32, tag="s1")
            nc.scalar.activation(s1[:], z1[:], ACT.Tanh, scale=inv1b[:, 0:1])
            a1 = wp.tile([128, NCH], FP32, tag="a1")
            nc.vector.tensor_tensor(out=a1[:], in0=s1[:], in1=kp1[:],
                                    op=OP.mult)
            km30 = wp.tile([128, NCH], FP32, tag="km30")
            nc.vector.tensor_scalar(km30[:], kp1[:], 1.0, BIG, OP.subtract,
                                    OP.mult)

            # g1 (scaled, masked transpose) + readout 1
            gmT1 = [bigp.tile([128, NPAD], BF16, tag=f"gmT1_{fc}",
                              name=f"gmT1_{fc}")
                    for fc in range(4)]
            ro1s_ps = psS.tile([1, HPAD], FP32, tag="rosum")
            for b in range(NCH):
                hc = h1[:, b * HPAD:(b + 1) * HPAD]
                g1c = wp.tile([128, HPAD], BF16, tag="g1c", bufs=2)
                nc.vector.tensor_scalar(g1c[:], hc, a1[:, b:b + 1], None,
                                        OP.mult)
                nc.tensor.matmul(out=ro1s_ps[:], lhsT=onesPb_t[:], rhs=g1c[:],
                                 start=(b == 0), stop=(b == NCH - 1))
                gmc = wp.tile([128, HPAD], BF16, tag="gmc", bufs=2)
                nc.vector.tensor_scalar(gmc[:], hc, a1[:, b:b + 1],
                                        km30[:, b:b + 1], OP.mult, OP.add)
                for fc in range(4):
                    tps = psT.tile([128, 128], BF16, tag="tps")
                    nc.tensor.transpose(out=tps[:],
                                        in_=gmc[:, fc * 128:(fc + 1) * 128],
                                        identity=idb_t[:])
                    nc.scalar.activation(gmT1[fc][:, b * 128:(b + 1) * 128],
                                         tps[:], ACT.Copy)
            m1T = wp.tile([128, 4], FP32, tag="m1T")
            for fc in range(4):
                nc.vector.tensor_reduce(out=m1T[:, fc:fc + 1], in_=gmT1[fc][:],
                                        op=OP.max, axis=AX.X)
            ro1s = wp.tile([1, HPAD], FP32, tag="ro1s")
            nc.vector.tensor_copy(ro1s[:], ro1s_ps[:])
            # ro1 rides in the z2ro payload (rows NPAD.. and NPAD+512..)
            nc.sync.dma_start(z2ro[NPAD:NPAD + HPAD, :]
                              .rearrange("f o -> o f"), ro1s[:])
            nc.sync.dma_start(
                z2ro[NPAD + HPAD:NPAD + 1024, :]
                .rearrange("(c p) o -> p (c o)", p=128), m1T[:])

            # ======================= layer 2 ===============================
            h2 = bigp.tile([128, NCH * HPAD], BF16, tag="h2_all")
            z2 = wp.tile([128, NCH], FP32, tag="z2")

            for b in range(NCH):
                agg_ps = psA.tile([128, HPAD], FP32, tag="aggps")
                for k in range(NB):
                    B = b * NB + k
                    gt = gp.tile([128, 8, ROWB], BF16, tag="gath", bufs=3)
                    nc.gpsimd.dma_gather(
                        gt[:], tblag[:], idx2_t[:, B * 64:(B + 1) * 64],
                        1024, 1024, ROWB)
                    wohb = sp.tile([128, 8, 128], BF16, tag="wohb2", bufs=6)
                    nc.sync.dma_start(
                        wohb[:].rearrange("p a d -> p (a d)"),
                        woh[:, B * 1024:(B + 1) * 1024])
                    # per-slot scale a1 = tanh(z*inv)*(z>=tau)
                    zg = gt[:, :, HPAD:HPAD + 2].bitcast(FP32) \
                        .rearrange("p a o -> p (a o)")
                    kp8 = wp.tile([128, 8], FP32, tag="kp8", bufs=2)
                    nc.vector.tensor_scalar(kp8[:], zg, tau1b[:, 0:1], None,
                                            OP.is_ge)
                    s8 = wp.tile([128, 8], FP32, tag="s8", bufs=2)
                    nc.scalar.activation(s8[:], zg, ACT.Tanh,
                                         scale=inv1b[:, 0:1])
                    a1s = wp.tile([128, 8], BF16, tag="a1s", bufs=2)
                    nc.vector.tensor_tensor(out=a1s[:], in0=s8[:], in1=kp8[:],
                                            op=OP.mult)
                    ohs = wp.tile([128, 8, 128], BF16, tag="ohs", bufs=2)
                    nc.vector.tensor_tensor(
                        out=ohs[:], in0=wohb[:],
                        in1=a1s[:].unsqueeze(2).broadcast_to([128, 8, 128]),
                        op=OP.mult)
                    for j in range(8):
                        nc.tensor.matmul(
                            out=agg_ps[:], lhsT=ohs[:, j, :],
                            rhs=gt[:, j, 0:HPAD],
                            start=(k == 0 and j == 0),
                            stop=(k == NB - 1 and j == 7))
                aggc = wp.tile([128, HPAD], BF16, tag="aggc2", bufs=2)
                nc.scalar.activation(aggc[:], agg_ps[:], ACT.Copy)
                for fc in range(4):
                    tps = psT.tile([128, 128], BF16, tag="tps")
                    nc.tensor.transpose(out=tps[:],
                                        in_=aggc[:, fc * 128:(fc + 1) * 128],
                                        identity=idb_t[:])
                    nc.scalar.activation(aggT2[fc][:, b * 128:(b + 1) * 128],
                                         tps[:], ACT.Copy)
                hp = psB.tile([128, HPAD], FP32, tag="hps")
                nc.tensor.matmul(out=hp[:], lhsT=ones1b_t[:], rhs=b2row_t[:],
                                 start=True, stop=False)
                for fc in range(4):
                    nc.tensor.matmul(
                        out=hp[:], lhsT=aggT2[fc][:, b * 128:(b + 1) * 128],
                        rhs=w2rel_t[fc][:], start=False, stop=False)
                for fc in range(4):
                    nc.tensor.matmul(
                        out=hp[:], lhsT=gmT1[fc][:, b * 128:(b + 1) * 128],
                        rhs=w2root_t[fc][:], start=False, stop=(fc == 3))
                hc = h2[:, b * HPAD:(b + 1) * HPAD]
                nc.scalar.activation(hc, hp[:], ACT.Relu)
                scr = wp.tile([128, HPAD], FP32, tag="scr", bufs=2)
                nc.vector.tensor_tensor(out=scr[:], in0=hc, in1=p2r_t[:],
                                        op=OP.mult)
                nc.vector.tensor_reduce(out=z2[:, b:b + 1], in_=scr[:],
                                        op=OP.add, axis=AX.X)

            # masked z2 (kept-in-l1 only) -> z2ro payload -> AllGather
            zm2 = wp.tile([128, NCH], FP32, tag="zm2")
            nc.vector.tensor_tensor(out=zm2[:], in0=z2[:], in1=kp1[:],
                                    op=OP.mult)
            nc.vector.tensor_tensor(out=zm2[:], in0=zm2[:], in1=km30[:],
                                    op=OP.add)
            nc.sync.dma_start(
                z2ro[0:NPAD, :].rearrange("(b p) o -> p (b o)", p=128),
                zm2[:])
            nc.gpsimd.collective_compute(
                "AllGather", OP.bypass, replica_groups=RG,
                ins=[z2ro[:]], outs=[z2roag[:]])

            # tau2 over the masked flat payload
            ztr = wp.tile([128, NF2], FP32, tag="ztr")
            nc.sync.dma_start(
                ztr[:], z2roag[:].rearrange("(p f) o -> p (f o)", p=128))
            zt2 = wp.tile([128, NF2], FP32, tag="zt2")
            nc.vector.tensor_tensor(out=zt2[:], in0=ztr[:], in1=rom_t[:],
                                    op=OP.mult)
            rm30 = wp.tile([128, NF2], FP32, tag="rm30")
            nc.vector.tensor_scalar(rm30[:], rom_t[:], 1.0, BIG, OP.subtract,
                                    OP.mult)
            nc.vector.tensor_tensor(out=zt2[:], in0=zt2[:], in1=rm30[:],
                                    op=OP.add)
            tau2b = topk_tau(zt2, NF2, K2, "l2")

            kp2 = wp.tile([128, NCH], FP32, tag="kp2")
            nc.vector.tensor_scalar(kp2[:], zm2[:], tau2b[:, 0:1], None,
                                    OP.is_ge)
            s2 = wp.tile([128, NCH], FP32, tag="s2")
            nc.scalar.activation(s2[:], z2[:], ACT.Tanh, scale=inv2b[:, 0:1])
            a2 = wp.tile([128, NCH], FP32, tag="a2")
            nc.vector.tensor_tensor(out=a2[:], in0=s2[:], in1=kp2[:],
                                    op=OP.mult)
            km30b = wp.tile([128, NCH], FP32, tag="km30b")
            nc.vector.tensor_scalar(km30b[:], kp2[:], 1.0, BIG, OP.subtract,
                                    OP.mult)

            ro2s_ps = psS.tile([1, HPAD], FP32, tag="rosum")
            m2T = wp.tile([128, 4], FP32, tag="m2T")
            nc.vector.memset(m2T[:], -1e30)
            for b in range(NCH):
                hc = h2[:, b * HPAD:(b + 1) * HPAD]
                g2c = wp.tile([128, HPAD], BF16, tag="g1c", bufs=2)
                nc.vector.tensor_scalar(g2c[:], hc, a2[:, b:b + 1], None,
                                        OP.mult)
                nc.tensor.matmul(out=ro2s_ps[:], lhsT=onesPb_t[:], rhs=g2c[:],
                                 start=(b == 0), stop=(b == NCH - 1))
                gmc = wp.tile([128, HPAD], BF16, tag="gmc", bufs=2)
                nc.vector.tensor_scalar(gmc[:], hc, a2[:, b:b + 1],
                                        km30b[:, b:b + 1], OP.mult, OP.add)
                for fc in range(4):
                    tps = psT.tile([128, 128], BF16, tag="tps")
                    nc.tensor.transpose(out=tps[:],
                                        in_=gmc[:, fc * 128:(fc + 1) * 128],
                                        identity=idb_t[:])
                    red = wp.tile([128, 1], FP32, tag="redm", bufs=2)
                    nc.vector.tensor_reduce(out=red[:], in_=tps[:],
                                            op=OP.max, axis=AX.X)
                    nc.vector.tensor_tensor(out=m2T[:, fc:fc + 1],
                                            in0=m2T[:, fc:fc + 1],
                                            in1=red[:], op=OP.max)
            ro2s = wp.tile([1, HPAD], FP32, tag="ro2s")
            nc.vector.tensor_copy(ro2s[:], ro2s_ps[:])
            nc.sync.dma_start(ro2in[0:1, :], ro2s[:])
            nc.sync.dma_start(
                ro2in[1:2, :].rearrange("o (c p) -> p (o c)", p=128), m2T[:])
            nc.gpsimd.collective_compute(
                "AllGather", OP.bypass, replica_groups=RG,
                ins=[ro2in[:]], outs=[ro2ag[:]])

            # ======================= readout combine + head ================
            # ro1 lives in z2roag rows [s*ZRO+NPAD, s*ZRO+NPAD+1024)
            mx1 = wp.tile([128, 4], FP32, tag="mx1")
            mn1 = wp.tile([128, 4], FP32, tag="mn1")
            sums1 = wp.tile([128, 4, NCORES], FP32, tag="cmb1")
            maxs1 = wp.tile([128, 4, NCORES], FP32, tag="cmbm1")
            for s in range(NCORES):
                base = s * ZRO + NPAD
                nc.sync.dma_start(
                    sums1[:, :, s],
                    z2roag[base:base + HPAD, :]
                    .rearrange("(c p) o -> p (c o)", p=128))
                nc.sync.dma_start(
                    maxs1[:, :, s],
                    z2roag[base + HPAD:base + 1024, :]
                    .rearrange("(c p) o -> p (c o)", p=128))
            nc.vector.tensor_reduce(out=mn1[:], in_=sums1[:], op=OP.add,
                                    axis=AX.X)
            nc.vector.tensor_reduce(out=mx1[:], in_=maxs1[:], op=OP.max,
                                    axis=AX.X)
            nc.vector.tensor_scalar_mul(mn1[:], mn1[:], 1.0 / K1)

            mx2 = wp.tile([128, 4], FP32, tag="mx2")
            mn2 = wp.tile([128, 4], FP32, tag="mn2")
            sums2 = wp.tile([128, 4, 2 * NCORES], FP32, tag="cmb2")
            for r in range(2 * NCORES):
                nc.sync.dma_start(
                    sums2[:, :, r],
                    ro2ag[r:r + 1, :].rearrange("o (c p) -> p (o c)", p=128))
            s_ap = sums2[:].rearrange("p c (s t) -> p c t s", t=2)
            nc.vector.tensor_reduce(out=mn2[:], in_=s_ap[:, :, 0, :],
                                    op=OP.add, axis=AX.X)
            nc.vector.tensor_reduce(out=mx2[:], in_=s_ap[:, :, 1, :],
                                    op=OP.max, axis=AX.X)
            nc.vector.tensor_scalar_mul(mn2[:], mn2[:], 1.0 / K2)

            zT = wp.tile([128, 8], FP32, tag="zT")
            nc.vector.tensor_tensor(out=zT[:, 0:4], in0=mx1[:], in1=mx2[:],
                                    op=OP.add)
            nc.vector.tensor_tensor(out=zT[:, 4:8], in0=mn1[:], in1=mn2[:],
                                    op=OP.add)
            zTb = wp.tile([128, 8], BF16, tag="zTb")
            nc.vector.tensor_copy(zTb[:], zT[:])

            # lin1 replicated: z1h [1, 2048] via psum-row matmuls
            b1h_t = load(b1h, BF16)
            qt = [psA.tile([128, HPAD], FP32, tag="aggps", name=f"hq{q}")
                  if q < 2 else
                  psB.tile([128, HPAD], FP32, tag="hps", name=f"hq{q}")
                  for q in range(4)]
            for q in range(4):
                nc.tensor.matmul(out=qt[q][0:1, :], lhsT=ones11_t[:],
                                 rhs=b1h_t[:, q * 512:(q + 1) * 512],
                                 start=True, stop=False, skip_group_check=True)
            for t in range(8):
                l1c = sp.tile([128, 2048], BF16, tag="l1s", bufs=2)
                nc.sync.dma_start(l1c[:], l1T[t * 128:(t + 1) * 128, :])
                for q in range(4):
                    nc.tensor.matmul(
                        out=qt[q][0:1, :], lhsT=zTb[:, t:t + 1],
                        rhs=l1c[:, q * 512:(q + 1) * 512],
                        start=False, stop=(t == 7), skip_group_check=True)
            z1h = wp.tile([1, 2048], BF16, tag="z1h")
            for q in range(4):
                nc.scalar.activation(z1h[:, q * 512:(q + 1) * 512],
                                     qt[q][0:1, :], ACT.Relu)
            z1hT = wp.tile([128, 16], BF16, tag="z1hT")
            for t in range(16):
                tpv = psT.tile([128, 1], BF16, tag="tps")
                nc.tensor.transpose(out=tpv[:],
                                    in_=z1h[:, t * 128:(t + 1) * 128],
                                    identity=ones11_t[:])
                nc.scalar.activation(z1hT[:, t:t + 1], tpv[:], ACT.Copy)

            # lin2 shard (500 rows), same psum-row form
            b2h_t = load(b2h, BF16)
            o2p = psA.tile([128, HPAD], FP32, tag="aggps")
            nc.tensor.matmul(out=o2p[0:1, 0:512], lhsT=ones11_t[:],
                             rhs=b2h_t[:], start=True, stop=False,
                             skip_group_check=True)
            for t in range(16):
                l2c = sp.tile([128, 500], BF16, tag="l2s")
                nc.sync.dma_start(l2c[:], l2T[t * 128:(t + 1) * 128, :])
                nc.tensor.matmul(out=o2p[0:1, 0:500], lhsT=z1hT[:, t:t + 1],
                                 rhs=l2c[:], start=False, stop=(t == 15),
                                 skip_group_check=True)
            z2h = wp.tile([1, HPAD], BF16, tag="z2h")
            nc.vector.memset(z2h[:], 0.0)
            nc.scalar.activation(z2h[:, 0:500], o2p[0:1, 0:500], ACT.Relu)
            z2hT = wp.tile([128, 4], BF16, tag="z2hT")
            for t in range(4):
                tpv = psT.tile([128, 1], BF16, tag="tps")
                nc.tensor.transpose(out=tpv[:],
                                    in_=z2h[:, t * 128:(t + 1) * 128],
                                    identity=ones11_t[:])
                nc.scalar.activation(z2hT[:, t:t + 1], tpv[:], ACT.Copy)

            # lin3 partial (own contraction shard) + AllReduce
            l3_t = load_chunks(l3T, 4, 128, "l3Tc")
            b3h_t = load(b3h)
            o3p = psB.tile([128, 1], FP32, tag="hps")
            for t in range(4):
                nc.tensor.matmul(out=o3p[:], lhsT=l3_t[t][:],
                                 rhs=z2hT[:, t:t + 1],
                                 start=(t == 0), stop=(t == 3))
            o3 = wp.tile([128, 1], FP32, tag="o3")
            nc.vector.tensor_copy(o3[:], o3p[:])
            nc.sync.dma_start(oin[:], o3[:])
            nc.gpsimd.collective_compute(
                "AllReduce", OP.add, replica_groups=RG,
                ins=[oin[:]], outs=[oar[:]])
            fin = wp.tile([128, 1], FP32, tag="fin")
            nc.sync.dma_start(fin[:], oar[:])
            nc.scalar.activation(fin[:], fin[:], ACT.Sigmoid,
                                 bias=b3h_t[:, 0:1])
            nc.sync.dma_start(out[:].rearrange("o f -> f o"), fin[:NOUT, :])

    nc.compile()
    return nc


# ---------------------------------------------------------------------------
# entry point
# ---------------------------------------------------------------------------

_CACHE = {}
TRACE = False


def kernel(**inputs):
    prep = _pack(inputs["x"], inputs["edge_src"], inputs["edge_dst"],
                 inputs["edge_weight"])
    if "nc" not in _CACHE:
        _CACHE["nc"] = _build()
    nc = _CACHE["nc"]
    in_maps = _host_inputs(inputs, prep)
    res = bass_utils.run_bass_kernel_spmd(
        nc, in_maps, core_ids=list(range(NCORES)), trace=TRACE)
    kernel.last_results = res
    return res.results[0]["out"]


if __name__ == "__main__":
    dat = np.load("/tmp/inputs.npz")
    inputs = {k: dat[k] for k in dat.files}
    got = kernel(**inputs)
    exp = np.load("/tmp/expected.npy")
    err = np.abs(got - exp).max()
    rel = err / np.abs(exp).max()
    print("out[0,:6] =", got[0, :6])
    print("exp[0,:6] =", exp[0, :6])
    print("max abs err:", err, "rel:", rel)


# revision 37
# speedup vs baseline: 1.1254x; 1.0292x over previous
"""Trainium2 Bass kernel for nn_Net_48301202211072 (GNN message passing).

2-layer GraphConv + TopKPooling + readout + MLP head, sharded over 8
NeuronCores. Strategy (v2):

- Nodes (and incident edges, grouped by destination) are sharded across
  cores. Edges are packed column-major into 11 destination bins per core
  (<=128 nodes and <=2048 edges per bin), 2 blocks of 1024 edge slots
  per bin. Everything is bf16 except score/threshold arithmetic.
- Layer-1 aggregation: the host pre-arranges per-edge source rows (xe)
  and edge-weighted one-hot matrices (Woh); the device streams both and
  aggregates with plain PE matmuls (no dma_gather, no per-edge DVE).
- Layer-2 aggregation: each node's table row [h1 | z1] is AllGather'd
  right after conv1 (before topk), so the big collective and the tau1
  histogram overlap; dma_gather fetches rows per edge and the per-slot
  scale tanh(z/||p||)*(z>=tau)*w is applied via the one-hot weights.
- TopK threshold: replicated 4-stage 64-bin histogram over the
  AllGather'd scores (bf16 compares, exact-enough within tolerance).
- Readout: ones-matmul mean + masked-transpose max, combined via small
  AllGathers (ro1 rides in the z2 AllGather payload).
- Head: lin1 replicated, lin2 sharded by rows, lin3 by contraction,
  one final AllReduce.
"""
import math
import sys

import numpy as np
import ml_dtypes

sys.path.insert(0, "/opt/trn_rl_repo")

import concourse.bacc as bacc  # noqa: E402
import concourse.mybir as mybir  # noqa: E402
import concourse.tile as tile  # noqa: E402
from concourse import bass_utils  # noqa: E402

FP32 = mybir.dt.float32
BF16 = mybir.dt.bfloat16
I16 = mybir.dt.int16
AX = mybir.AxisListType
OP = mybir.AluOpType
ACT = mybir.ActivationFunctionType
BFNP = ml_dtypes.bfloat16

NCORES = 8
N = 10000
FIN = 256
HID = 500
HPAD = 512
NOUT = 100
NPC = N // NCORES          # 1250 nodes per core
NCH = 11                   # dst bins per core (<=128 nodes, <=2048 edges)
NB = 2                     # blocks of 1024 edge slots per bin
BTOT = NCH * NB            # 22
NPAD = NCH * 128           # 1408 table rows per core
NROWS = NCORES * NPAD      # 11264
ROWB = 640                 # table row: 512 h bf16 + 2 z-as-bf16 + 126 pad
NBINS = 32
NSTAGES = 3
K1 = N // 2
K2 = N // 4
ZRO = NPAD + 1024          # 2432: zm2 + ro1 payload rows per core
SPLITB = 6                 # bins 0..5 AllGather'd early (rows 0:768)
SPLITR = SPLITB * 128      # 768
ASZ = NCORES * SPLITR      # 6144 rows in part A of the gathered table
BIG = 1e30


# ---------------------------------------------------------------------------
# host preprocessing
# ---------------------------------------------------------------------------

def _pack(x, edge_src, edge_dst, edge_weight):
    src = np.asarray(edge_src, np.int64)
    dst = np.asarray(edge_dst, np.int64)
    w = np.asarray(edge_weight, np.float32)
    x_bf = np.ascontiguousarray(np.asarray(x, np.float32)).astype(BFNP)

    # pass 1: per-core greedy bin boundaries + node->table-row map
    binrow = np.zeros((NCORES, NPC), np.int64)   # local node -> row in [0,NPAD)
    starts_all, counts_all = [], []
    for c in range(NCORES):
        lo = c * NPC
        m = (dst >= lo) & (dst < lo + NPC)
        ed = dst[m] - lo
        deg = np.bincount(ed, minlength=NPC)
        bstart, bnodes = [], []
        n0 = 0
        while n0 < NPC:
            e_acc, nn = 0, 0
            while n0 + nn < NPC and nn < 128 and e_acc + deg[n0 + nn] <= 2048:
                e_acc += deg[n0 + nn]
                nn += 1
            assert nn > 0
            bstart.append(n0)
            bnodes.append(nn)
            n0 += nn
        assert len(bstart) <= NCH, f"core {c} needs {len(bstart)} bins"
        while len(bstart) < NCH:
            bstart.append(NPC)
            bnodes.append(0)
        bstart = np.asarray(bstart, np.int64)
        bnodes = np.asarray(bnodes, np.int64)
        for b in range(NCH):
            s, nn = bstart[b], bnodes[b]
            binrow[c, s:s + nn] = b * 128 + np.arange(nn)
        starts_all.append(bstart)
        counts_all.append(bnodes)

    per_core = []
    for c in range(NCORES):
        lo = c * NPC
        m = (dst >= lo) & (dst < lo + NPC)
        es, ed, ew = src[m], dst[m] - lo, w[m]
        order = np.argsort(ed, kind="stable")
        es, ed, ew = es[order], ed[order], ew[order]
        bstart, bnodes = starts_all[c], counts_all[c]
        # edge ranges per bin (bins are consecutive node ranges)
        bin_edge_start = np.searchsorted(ed, bstart)
        bin_edge_end = np.searchsorted(ed, bstart + bnodes)

        # slot assignment (column-major within each bin's 2 blocks)
        srcslot = np.full(BTOT * 1024, -1, np.int64)
        dslot = np.zeros(BTOT * 1024, np.int64)
        wslot = np.zeros(BTOT * 1024, np.float32)
        for b in range(NCH):
            e0, e1 = bin_edge_start[b], bin_edge_end[b]
            cnt = e1 - e0
            assert cnt <= NB * 1024
            u = np.arange(cnt)
            blk = b * NB + u // 1024
            u2 = u % 1024
            pos = blk * 1024 + u2
            srcslot[pos] = es[e0:e1]
            dslot[pos] = ed[e0:e1] - bstart[b]
            wslot[pos] = ew[e0:e1]

        real = srcslot >= 0
        # xe: [128, BTOT*8*256] pre-gathered source rows, bf16
        rows = np.zeros((BTOT * 1024, FIN), BFNP)
        rows[real] = x_bf[srcslot[real]]
        xe = np.ascontiguousarray(
            rows.reshape(BTOT, 8, 128, FIN).transpose(2, 0, 1, 3)
            .reshape(128, BTOT * 8 * FIN))

        # woh: [128, BTOT*8*128] edge-weighted one-hots, bf16
        woh = np.zeros((128, BTOT * 8 * 128), np.float32)
        pos = np.nonzero(real)[0]
        blk = pos // 1024
        u2 = pos % 1024
        p = u2 % 128
        j = u2 // 128
        woh[p, (blk * 8 + j) * 128 + dslot[pos]] = wslot[pos]
        woh = woh.astype(BFNP)

        # idx2: slot -> row in the AllGather'd table
        sc = srcslot // NPC
        slo = srcslot - sc * NPC
        idx2 = np.zeros(BTOT * 1024, np.int64)
        idx2[real] = sc[real] * NPAD + binrow[sc[real], slo[real]]
        per_core.append(dict(xe=xe, woh=woh, idx2=idx2,
                             bstart=bstart, bnodes=bnodes))
    return per_core


def _wrap16(idx_flat):
    """[BTOT*1024] -> [128, BTOT*64] int16, per-block wrapped-16 replicated."""
    out = np.zeros((128, BTOT * 64), np.int16)
    for blk in range(BTOT):
        b = idx_flat[blk * 1024:(blk + 1) * 1024].astype(np.int16)
        t = b.reshape(64, 16).T          # [16, 64]
        out[:, blk * 64:(blk + 1) * 64] = np.tile(t, (8, 1))
    return out


def _host_inputs(inputs, prep):
    x = np.ascontiguousarray(np.asarray(inputs["x"], np.float32))

    def padT(a, rows, cols):
        out = np.zeros((rows, cols), np.float32)
        t = np.asarray(a, np.float32).T
        out[: t.shape[0], : t.shape[1]] = t
        return out.astype(BFNP)

    w1relT = padT(inputs["W1_rel"], FIN, HPAD)
    w1rootT = padT(inputs["W1_root"], FIN, HPAD)
    w2relT = padT(inputs["W2_rel"], HPAD, HPAD)
    w2rootT = padT(inputs["W2_root"], HPAD, HPAD)

    def rowv(v):
        out = np.zeros((1, HPAD), np.float32)
        vv = np.asarray(v, np.float32)
        out[0, : vv.shape[0]] = vv
        return out

    b1row = rowv(inputs["b1"]).astype(BFNP)
    b2row = rowv(inputs["b2"]).astype(BFNP)
    p1f = rowv(inputs["p1_w"])
    p2f = rowv(inputs["p2_w"])
    p1r = np.tile(p1f, (128, 1)).astype(BFNP)
    p2r = np.tile(p2f, (128, 1)).astype(BFNP)

    iotaB = np.tile(np.arange(NBINS, dtype=np.float32)[None, :], (128, 1))
    identb = np.eye(128, dtype=np.float32).astype(BFNP)
    identf = np.eye(128, dtype=np.float32)
    ones128f = np.ones((1, 128), np.float32)
    ones1b = np.ones((1, 128), np.float32).astype(BFNP)
    onesPb = np.ones((128, 1), np.float32).astype(BFNP)
    ones11 = np.ones((1, 1), np.float32).astype(BFNP)

    # mask of z positions inside the flattened z2ro AllGather payload
    g = np.arange(128 * (NCORES * ZRO // 128), dtype=np.int64)
    romask = ((g % ZRO) < NPAD).astype(np.float32).reshape(
        128, NCORES * ZRO // 128)

    lin1W = np.asarray(inputs["lin1_W"], np.float32)   # [2000, 1000]
    lin2W = np.asarray(inputs["lin2_W"], np.float32)   # [4000, 2000]
    lin3W = np.asarray(inputs["lin3_W"], np.float32)   # [100, 4000]
    lin1b = np.asarray(inputs["lin1_b"], np.float32)
    lin2b = np.asarray(inputs["lin2_b"], np.float32)
    lin3b = np.asarray(inputs["lin3_b"], np.float32)

    # lin1 replicated: rows = z layout [max 0:500 | pad | mean 512:1012 | pad]
    l1T = np.zeros((1024, 2048), np.float32)
    sh = lin1W.T                                       # [1000, 2000]
    l1T[:500, :2000] = sh[:500]
    l1T[512:1012, :2000] = sh[500:]
    l1T = l1T.astype(BFNP)
    b1h = np.zeros((1, 2048), np.float32)
    b1h[0, :2000] = lin1b
    b1h = b1h.astype(BFNP)

    per_core = []
    for c in range(NCORES):
        pr = prep[c]
        bstart, bnodes = pr["bstart"], pr["bnodes"]

        xT = np.zeros((FIN, NPAD), np.float32)
        padmask = np.zeros((128, NCH), np.float32)
        for b in range(NCH):
            s, nn = bstart[b], bnodes[b]
            if nn:
                xT[:, b * 128: b * 128 + nn] = x[c * NPC + s: c * NPC + s + nn].T
                padmask[:nn, b] = 1.0
        xT = xT.astype(BFNP)

        l2T = np.zeros((2048, 500), np.float32)
        l2T[:2000] = lin2W[c * 500:(c + 1) * 500].T
        l2T = l2T.astype(BFNP)
        b2h = np.zeros((1, 512), np.float32)
        b2h[0, :500] = lin2b[c * 500:(c + 1) * 500]
        b2h = b2h.astype(BFNP)

        l3T = np.zeros((512, 128), np.float32)
        l3T[:500, :NOUT] = lin3W[:, c * 500:(c + 1) * 500].T
        l3T = l3T.astype(BFNP)
        b3h = np.zeros((128, 1), np.float32)
        b3h[:NOUT, 0] = lin3b

        per_core.append(dict(
            xe=pr["xe"], woh=pr["woh"], idx2=_wrap16(pr["idx2"]),
            padmask=padmask, xT=xT,
            w1relT=w1relT, w1rootT=w1rootT, w2relT=w2relT, w2rootT=w2rootT,
            b1row=b1row, b2row=b2row, p1f=p1f, p2f=p2f, p1r=p1r, p2r=p2r,
            iotaB=iotaB, identb=identb, identf=identf, ones128f=ones128f,
            ones1b=ones1b, onesPb=onesPb, ones11=ones11,
            romask=romask,
            l1T=l1T, b1h=b1h, l2T=l2T, b2h=b2h, l3T=l3T, b3h=b3h,
        ))
    return per_core


# ---------------------------------------------------------------------------
# device program
# ---------------------------------------------------------------------------

def _build():
    nc = bacc.Bacc("TRN2", target_bir_lowering=False, debug=False,
                   num_devices=NCORES)

    def din(name, shape, dt=FP32):
        return nc.dram_tensor(name, shape, dt, kind="ExternalInput")

    xe = din("xe", [128, BTOT * 8 * FIN], BF16)
    woh = din("woh", [128, BTOT * 8 * 128], BF16)
    idx2 = din("idx2", [128, BTOT * 64], I16)
    padmask = din("padmask", [128, NCH])
    xT = din("xT", [FIN, NPAD], BF16)
    w1relT = din("w1relT", [FIN, HPAD], BF16)
    w1rootT = din("w1rootT", [FIN, HPAD], BF16)
    w2relT = din("w2relT", [HPAD, HPAD], BF16)
    w2rootT = din("w2rootT", [HPAD, HPAD], BF16)
    b1row = din("b1row", [1, HPAD], BF16)
    b2row = din("b2row", [1, HPAD], BF16)
    p1f = din("p1f", [1, HPAD])
    p2f = din("p2f", [1, HPAD])
    p1r = din("p1r", [128, HPAD], BF16)
    p2r = din("p2r", [128, HPAD], BF16)
    iotaB = din("iotaB", [128, NBINS])
    identb = din("identb", [128, 128], BF16)
    identf = din("identf", [128, 128])
    ones128f = din("ones128f", [1, 128])
    ones1b = din("ones1b", [1, 128], BF16)
    onesPb = din("onesPb", [128, 1], BF16)
    ones11 = din("ones11", [1, 1], BF16)
    romask = din("romask", [128, NCORES * ZRO // 128])
    l1T = din("l1T", [1024, 2048], BF16)
    b1h = din("b1h", [1, 2048], BF16)
    l2T = din("l2T", [2048, 500], BF16)
    b2h = din("b2h", [1, 512], BF16)
    l3T = din("l3T", [512, 128], BF16)
    b3h = din("b3h", [128, 1])

    out = nc.dram_tensor("out", [1, NOUT], FP32, kind="ExternalOutput")

    RG = [list(range(NCORES))]

    with tile.TileContext(nc) as tc:
        with (
            tc.tile_pool(name="const", bufs=1) as cp,
            tc.tile_pool(name="stream", bufs=3) as sp,
            tc.tile_pool(name="gather", bufs=2) as gp,
            tc.tile_pool(name="work", bufs=1) as wp,
            tc.tile_pool(name="big", bufs=1) as bigp,
            tc.tile_pool(name="psA", bufs=2, space="PSUM") as psA,
            tc.tile_pool(name="psB", bufs=2, space="PSUM") as psB,
            tc.tile_pool(name="psT", bufs=2, space="PSUM") as psT,
            tc.tile_pool(name="psS", bufs=1, space="PSUM") as psS,
            tc.tile_pool(name="dram", bufs=1, space="DRAM") as dr,
        ):
            def load(src, dt=FP32, tag=None):
                tl = cp.tile(list(src.shape), dt, tag=tag or src.name)
                nc.sync.dma_start(tl[:], src[:])
                return tl

            idx2_t = load(idx2, I16)
            pad_t = load(padmask)
            iob_t = load(iotaB)
            idb_t = load(identb, BF16)
            idf_t = load(identf)
            ones_t = load(ones128f)
            ones1b_t = load(ones1b, BF16)
            onesPb_t = load(onesPb, BF16)
            ones11_t = load(ones11, BF16)
            b1row_t = load(b1row, BF16)
            b2row_t = load(b2row, BF16)
            p1f_t = load(p1f)
            p2f_t = load(p2f)
            p1r_t = load(p1r, BF16)
            p2r_t = load(p2r, BF16)
            rom_t = load(romask)

            def load_chunks(src, nchunks, cols, tag, dt=BF16):
                ts = []
                for k in range(nchunks):
                    t = cp.tile([128, cols], dt, tag=f"{tag}{k}")
                    nc.sync.dma_start(t[:], src[k * 128:(k + 1) * 128, :cols])
                    ts.append(t)
                return ts

            w1rel_t = load_chunks(w1relT, 2, HPAD, "w1rel")
            w1root_t = load_chunks(w1rootT, 2, HPAD, "w1root")
            w2rel_t = load_chunks(w2relT, 4, HPAD, "w2rel")
            w2root_t = load_chunks(w2rootT, 4, HPAD, "w2root")
            xT_t = load_chunks(xT, 2, NPAD, "xTc")

            # DRAM internal tiles
            tbl = dr.tile([NPAD, ROWB], BF16)
            tblag = dr.tile([NROWS, ROWB], BF16, addr_space="Shared")
            zsh1 = dr.tile([NPAD, 1], FP32)
            zag1 = dr.tile([NROWS, 1], FP32, addr_space="Shared")
            z2ro = dr.tile([ZRO, 1], FP32)
            z2roag = dr.tile([NCORES * ZRO, 1], FP32, addr_space="Shared")
            ro2in = dr.tile([2, HPAD], FP32)
            ro2ag = dr.tile([2 * NCORES, HPAD], FP32, addr_space="Shared")
            oin = dr.tile([128, 1], FP32)
            oar = dr.tile([128, 1], FP32, addr_space="Shared")
            wrm = dr.tile([16, 1], FP32)
            wrmag = dr.tile([16 * NCORES, 1], FP32, addr_space="Shared")

            # collective-stack warmup: absorb first-collective setup cost
            # while layer 1 computes
            wz = wp.tile([16, 1], FP32, tag="wz")
            nc.vector.memset(wz[:], 0.0)
            nc.sync.dma_start(wrm[:], wz[:])
            nc.gpsimd.collective_compute(
                "AllGather", OP.bypass, replica_groups=RG,
                ins=[wrm[:]], outs=[wrmag[:]])

            # -------- norms first (Sqrt table load hides under L1) ---------
            def inv_norm_b(pf_t, lname):
                """[128,1] broadcast of 1/||p||."""
                sq = wp.tile([1, HPAD], FP32, tag="pnsq")
                nc.vector.tensor_tensor(out=sq[:], in0=pf_t[:], in1=pf_t[:],
                                        op=OP.mult)
                n2 = wp.tile([1, 1], FP32, tag="pn2")
                nc.vector.tensor_reduce(out=n2[:], in_=sq[:], op=OP.add,
                                        axis=AX.X)
                nc.scalar.activation(n2[:], n2[:], ACT.Sqrt)
                nc.vector.reciprocal(n2[:], n2[:])
                ib_ps = psS.tile([128, 1], FP32, tag="small")
                nc.tensor.matmul(out=ib_ps[:], lhsT=ones_t[:], rhs=n2[:],
                                 start=True, stop=True)
                ib = wp.tile([128, 1], FP32, tag=f"invbs{lname}")
                nc.vector.tensor_copy(ib[:], ib_ps[:])
                return ib

            inv1b = inv_norm_b(p1f_t, "l1")
            inv2b = inv_norm_b(p2f_t, "l2")

            # ---------------- histogram k-th threshold ---------------------
            NF2 = NCORES * ZRO // 128
            S_big = wp.tile([128, NF2 * NBINS], BF16, tag="Sbig")

            def topk_tau(zt, nfree, k, lname):
                """zt: [128, nfree] fp32 scores (pads/masked = -BIG).
                returns [128,1] tile with the k-th-largest threshold."""
                mm = wp.tile([128, 2], FP32, tag="mm")
                msk = wp.tile([128, nfree], FP32, tag=f"hmsk{lname}")
                nc.vector.tensor_scalar(msk[:], zt[:], -1e29, 2e30, OP.is_lt,
                                        OP.mult)
                nc.vector.tensor_tensor(out=msk[:], in0=msk[:], in1=zt[:],
                                        op=OP.add)
                nc.vector.tensor_reduce(out=mm[:, 0:1], in_=msk[:], op=OP.min,
                                        axis=AX.X)
                nc.vector.tensor_reduce(out=mm[:, 1:2], in_=zt[:], op=OP.max,
                                        axis=AX.X)
                ztb = wp.tile([128, nfree], BF16, tag=f"ztb{lname}")
                nc.vector.tensor_copy(ztb[:], zt[:])
                lw = wp.tile([1, 2], FP32, tag="lw")  # [lo, w]
                mmT = wp.tile([1, 2, 128], FP32, tag="mmTs")
                for col in range(2):
                    mmT_ps = psS.tile([1, 128], FP32, tag="small")
                    nc.tensor.transpose(out=mmT_ps[:], in_=mm[:, col:col + 1],
                                        identity=idf_t[:])
                    nc.vector.tensor_copy(mmT[:, col, :], mmT_ps[:])
                nc.vector.tensor_reduce(out=lw[:, 0:1], in_=mmT[:, 0, :],
                                        op=OP.min, axis=AX.X)
                nc.vector.tensor_reduce(out=lw[:, 1:2], in_=mmT[:, 1, :],
                                        op=OP.max, axis=AX.X)
                nc.vector.tensor_scalar_add(lw[:, 0:1], lw[:, 0:1], -1e-3)
                nc.vector.tensor_scalar_add(lw[:, 1:2], lw[:, 1:2], 1e-3)
                nc.vector.tensor_tensor(out=lw[:, 1:2], in0=lw[:, 1:2],
                                        in1=lw[:, 0:1], op=OP.subtract)
                nc.vector.tensor_scalar_mul(lw[:, 1:2], lw[:, 1:2], 1.0 / NBINS)

                for st in range(NSTAGES):
                    lwb_ps = psS.tile([128, 2], FP32, tag="small")
                    nc.tensor.matmul(out=lwb_ps[:], lhsT=ones_t[:], rhs=lw[:],
                                     start=True, stop=True)
                    lwb = wp.tile([128, 2], FP32, tag="lwbs")
                    nc.vector.tensor_copy(lwb[:], lwb_ps[:])
                    tt = wp.tile([128, NBINS], FP32, tag="tt")
                    nc.vector.tensor_scalar(tt[:], iob_t[:], lwb[:, 1:2],
                                            lwb[:, 0:1], OP.mult, OP.add)
                    ttb = wp.tile([128, NBINS], BF16, tag="ttb")
                    nc.vector.tensor_copy(ttb[:], tt[:])
                    # S[p, j, n]: count-reduce over n is contiguous
                    S = S_big[:, :NBINS * nfree].rearrange(
                        "p (j n) -> p j n", j=NBINS)
                    nc.vector.tensor_tensor(
                        out=S,
                        in0=ztb[:].unsqueeze(1).broadcast_to(
                            [128, NBINS, nfree]),
                        in1=ttb[:].unsqueeze(2).broadcast_to(
                            [128, NBINS, nfree]),
                        op=OP.is_ge)
                    cntp = wp.tile([128, NBINS], BF16, tag="cntp")
                    with nc.allow_low_precision(
                            reason="counts <= nfree are exact in bf16"):
                        nc.vector.tensor_reduce(
                            out=cntp[:], in_=S, op=OP.add, axis=AX.X)
                    cnt_ps = psS.tile([1, NBINS], FP32, tag="small")
                    nc.tensor.matmul(out=cnt_ps[:], lhsT=onesPb_t[:],
                                     rhs=cntp[:], start=True, stop=True)
                    fl = wp.tile([1, NBINS], FP32, tag="fl")
                    nc.vector.tensor_scalar(fl[:], cnt_ps[:], float(k), None,
                                            OP.is_ge)
                    js = wp.tile([1, 1], FP32, tag="js")
                    nc.vector.tensor_reduce(out=js[:], in_=fl[:], op=OP.add,
                                            axis=AX.X)
                    nc.vector.tensor_scalar_add(js[:], js[:], -1.0)
                    nc.vector.tensor_scalar(lw[:, 0:1], js[:], lw[:, 1:2],
                                            lw[:, 0:1], OP.mult, OP.add)
                    if st != NSTAGES - 1:
                        nc.vector.tensor_scalar_mul(lw[:, 1:2], lw[:, 1:2],
                                                    1.0 / NBINS)
                taub_ps = psS.tile([128, 1], FP32, tag="small")
                nc.tensor.matmul(out=taub_ps[:], lhsT=ones_t[:],
                                 rhs=lw[:, 0:1], start=True, stop=True)
                taub = wp.tile([128, 1], FP32, tag=f"taubs{lname}")
                nc.vector.tensor_copy(taub[:], taub_ps[:])
                return taub

            # ======================= layer 1 ===============================
            h1 = bigp.tile([128, NCH * HPAD], BF16, tag="h1_all")
            z1 = wp.tile([128, NCH], FP32, tag="z1")
            # aggT tiles shared between layers (L1 uses the first two; its
            # dense reads complete before L2 overwrites them)
            aggT2 = [bigp.tile([128, NPAD], BF16, tag=f"aggT2_{fc}",
                               name=f"aggT2_{fc}")
                     for fc in range(4)]
            aggT1 = aggT2[:2]

            for b in range(NCH):
                agg_ps = psA.tile([128, HPAD], FP32, tag="aggps")
                for k in range(NB):
                    B = b * NB + k
                    xeb = sp.tile([128, 8, FIN], BF16, tag="xeb", bufs=2)
                    nc.sync.dma_start(
                        xeb[:].rearrange("p a f -> p (a f)"),
                        xe[:, B * 8 * FIN:(B + 1) * 8 * FIN])
                    wohb = sp.tile([128, 8, 128], BF16, tag="wohb1")
                    nc.sync.dma_start(
                        wohb[:].rearrange("p a d -> p (a d)"),
                        woh[:, B * 1024:(B + 1) * 1024])
                    for j in range(8):
                        nc.tensor.matmul(
                            out=agg_ps[:, :FIN], lhsT=wohb[:, j, :],
                            rhs=xeb[:, j, :],
                            start=(k == 0 and j == 0),
                            stop=(k == NB - 1 and j == 7))
                # transpose agg -> aggT1 chunks
                aggc = wp.tile([128, FIN], BF16, tag="aggc", bufs=2)
                nc.scalar.activation(aggc[:], agg_ps[:, :FIN], ACT.Copy)
                for fc in range(2):
                    tps = psT.tile([128, 128], BF16, tag="tps")
                    nc.tensor.transpose(out=tps[:],
                                        in_=aggc[:, fc * 128:(fc + 1) * 128],
                                        identity=idb_t[:])
                    nc.scalar.activation(aggT1[fc][:, b * 128:(b + 1) * 128],
                                         tps[:], ACT.Copy)
                # dense: h = relu(b1 + aggT.T @ w1relT + xT.T @ w1rootT)
                hp = psB.tile([128, HPAD], FP32, tag="hps")
                nc.tensor.matmul(out=hp[:], lhsT=ones1b_t[:], rhs=b1row_t[:],
                                 start=True, stop=False)
                for fc in range(2):
                    nc.tensor.matmul(
                        out=hp[:], lhsT=aggT1[fc][:, b * 128:(b + 1) * 128],
                        rhs=w1rel_t[fc][:], start=False, stop=False)
                for fc in range(2):
                    nc.tensor.matmul(
                        out=hp[:], lhsT=xT_t[fc][:, b * 128:(b + 1) * 128],
                        rhs=w1root_t[fc][:], start=False, stop=(fc == 1))
                hc = h1[:, b * HPAD:(b + 1) * HPAD]
                nc.scalar.activation(hc, hp[:], ACT.Relu)
                # z score (fp32)
                scr = wp.tile([128, HPAD], BF16, tag="scr", bufs=2)
                nc.vector.tensor_tensor(out=scr[:], in0=hc, in1=p1r_t[:],
                                        op=OP.mult)
                nc.vector.tensor_reduce(out=z1[:, b:b + 1], in_=scr[:],
                                        op=OP.add, axis=AX.X)
                # table row: [h | z | pad]
                tblb = wp.tile([128, ROWB], BF16, tag="tblb", bufs=2)
                nc.scalar.activation(tblb[:, 0:HPAD], hp[:], ACT.Relu)
                nc.vector.tensor_copy(
                    tblb[:, HPAD:HPAD + 2].bitcast(FP32), z1[:, b:b + 1])
                nc.sync.dma_start(tbl[b * 128:(b + 1) * 128, :], tblb[:])

            # masked z for selection
            pm30 = wp.tile([128, NCH], FP32, tag="pm30")
            nc.vector.tensor_scalar(pm30[:], pad_t[:], 1.0, BIG, OP.subtract,
                                    OP.mult)
            zm1 = wp.tile([128, NCH], FP32, tag="zm1")
            nc.vector.tensor_tensor(out=zm1[:], in0=z1[:], in1=pad_t[:],
                                    op=OP.mult)
            nc.vector.tensor_tensor(out=zm1[:], in0=zm1[:], in1=pm30[:],
                                    op=OP.add)
            nc.sync.dma_start(
                zsh1[:].rearrange("(b p) o -> p (b o)", p=128), zm1[:])
            nc.gpsimd.collective_compute(
                "AllGather", OP.bypass, replica_groups=RG,
                ins=[tbl[:]], outs=[tblag[:]])
            nc.gpsimd.collective_compute(
                "AllGather", OP.bypass, replica_groups=RG,
                ins=[zsh1[:]], outs=[zag1[:]])

            zt1 = wp.tile([128, NROWS // 128], FP32, tag="zt1")
            nc.sync.dma_start(
                zt1[:], zag1[:].rearrange("(p f) o -> p (f o)", p=128))
            tau1b = topk_tau(zt1, NROWS // 128, K1, "l1")

            # a1 per local bin + kept masks
            kp1 = wp.tile([128, NCH], FP32, tag="kp1")
            nc.vector.tensor_scalar(kp1[:], zm1[:], tau1b[:, 0:1], None,
                                    OP.is_ge)
            s1 = wp.tile([128, NCH], FP32, tag="s1")
            nc.scalar.activation(s1[:], z1[:], ACT.Tanh, scale=inv1b[:, 0:1])
            a1 = wp.tile([128, NCH], FP32, tag="a1")
            nc.vector.tensor_tensor(out=a1[:], in0=s1[:], in1=kp1[:],
                                    op=OP.mult)
            km30 = wp.tile([128, NCH], FP32, tag="km30")
            nc.vector.tensor_scalar(km30[:], kp1[:], 1.0, BIG, OP.subtract,
                                    OP.mult)

            # g1 (scaled, masked transpose) + readout 1
            gmT1 = [bigp.tile([128, NPAD], BF16, tag=f"gmT1_{fc}",
                              name=f"gmT1_{fc}")
                    for fc in range(4)]
            ro1s_ps = psS.tile([1, HPAD], FP32, tag="rosum")
            for b in range(NCH):
                hc = h1[:, b * HPAD:(b + 1) * HPAD]
                g1c = wp.tile([128, HPAD], BF16, tag="g1c", bufs=2)
                nc.vector.tensor_scalar(g1c[:], hc, a1[:, b:b + 1], None,
                                        OP.mult)
                nc.tensor.matmul(out=ro1s_ps[:], lhsT=onesPb_t[:], rhs=g1c[:],
                                 start=(b == 0), stop=(b == NCH - 1))
                gmc = wp.tile([128, HPAD], BF16, tag="gmc", bufs=2)
                nc.vector.tensor_scalar(gmc[:], hc, a1[:, b:b + 1],
                                        km30[:, b:b + 1], OP.mult, OP.add)
                for fc in range(4):
                    tps = psT.tile([128, 128], BF16, tag="tps")
                    nc.tensor.transpose(out=tps[:],
                                        in_=gmc[:, fc * 128:(fc + 1) * 128],
                                        identity=idb_t[:])
                    nc.scalar.activation(gmT1[fc][:, b * 128:(b + 1) * 128],
                                         tps[:], ACT.Copy)
            m1T = wp.tile([128, 4], FP32, tag="m1T")
            for fc in range(4):
                nc.vector.tensor_reduce(out=m1T[:, fc:fc + 1], in_=gmT1[fc][:],
                                        op=OP.max, axis=AX.X)
            ro1s = wp.tile([1, HPAD], FP32, tag="ro1s")
            nc.vector.tensor_copy(ro1s[:], ro1s_ps[:])
            # ro1 rides in the z2ro payload (rows NPAD.. and NPAD+512..)
            nc.sync.dma_start(z2ro[NPAD:NPAD + HPAD, :]
                              .rearrange("f o -> o f"), ro1s[:])
            nc.sync.dma_start(
                z2ro[NPAD + HPAD:NPAD + 1024, :]
                .rearrange("(c p) o -> p (c o)", p=128), m1T[:])

            # ======================= layer 2 ===============================
            h2 = bigp.tile([128, NCH * HPAD], BF16, tag="h2_all")
            z2 = wp.tile([128, NCH], FP32, tag="z2")

            for b in range(NCH):
                agg_ps = psA.tile([128, HPAD], FP32, tag="aggps")
                for k in range(NB):
                    B = b * NB + k
                    gt = gp.tile([128, 8, ROWB], BF16, tag="gath", bufs=3)
                    nc.gpsimd.dma_gather(
                        gt[:], tblag[:], idx2_t[:, B * 64:(B + 1) * 64],
                        1024, 1024, ROWB)
                    wohb = sp.tile([128, 8, 128], BF16, tag="wohb2", bufs=10)
                    nc.sync.dma_start(
                        wohb[:].rearrange("p a d -> p (a d)"),
                        woh[:, B * 1024:(B + 1) * 1024])
                    # per-slot scale a1 = tanh(z*inv)*(z>=tau)
                    zg = gt[:, :, HPAD:HPAD + 2].bitcast(FP32) \
                        .rearrange("p a o -> p (a o)")
                    kp8 = wp.tile([128, 8], FP32, tag="kp8", bufs=2)
                    nc.vector.tensor_scalar(kp8[:], zg, tau1b[:, 0:1], None,
                                            OP.is_ge)
                    s8 = wp.tile([128, 8], FP32, tag="s8", bufs=2)
                    nc.scalar.activation(s8[:], zg, ACT.Tanh,
                                         scale=inv1b[:, 0:1])
                    a1s = wp.tile([128, 8], BF16, tag="a1s", bufs=2)
                    nc.vector.tensor_tensor(out=a1s[:], in0=s8[:], in1=kp8[:],
                                            op=OP.mult)
                    ohs = wp.tile([128, 8, 128], BF16, tag="ohs", bufs=2)
                    nc.vector.tensor_tensor(
                        out=ohs[:], in0=wohb[:],
                        in1=a1s[:].unsqueeze(2).broadcast_to([128, 8, 128]),
                        op=OP.mult)
                    for j in range(8):
                        nc.tensor.matmul(
                            out=agg_ps[:], lhsT=ohs[:, j, :],
                            rhs=gt[:, j, 0:HPAD],
                            start=(k == 0 and j == 0),
                            stop=(k == NB - 1 and j == 7))
                aggc = wp.tile([128, HPAD], BF16, tag="aggc2", bufs=2)
                nc.scalar.activation(aggc[:], agg_ps[:], ACT.Copy)
                for fc in range(4):
                    tps = psT.tile([128, 128], BF16, tag="tps")
                    nc.tensor.transpose(out=tps[:],
                                        in_=aggc[:, fc * 128:(fc + 1) * 128],
                                        identity=idb_t[:])
                    nc.scalar.activation(aggT2[fc][:, b * 128:(b + 1) * 128],
                                         tps[:], ACT.Copy)
                hp = psB.tile([128, HPAD], FP32, tag="hps")
                nc.tensor.matmul(out=hp[:], lhsT=ones1b_t[:], rhs=b2row_t[:],
                                 start=True, stop=False)
                for fc in range(4):
                    nc.tensor.matmul(
                        out=hp[:], lhsT=aggT2[fc][:, b * 128:(b + 1) * 128],
                        rhs=w2rel_t[fc][:], start=False, stop=False)
                for fc in range(4):
                    nc.tensor.matmul(
                        out=hp[:], lhsT=gmT1[fc][:, b * 128:(b + 1) * 128],
                        rhs=w2root_t[fc][:], start=False, stop=(fc == 3))
                hc = h2[:, b * HPAD:(b + 1) * HPAD]
                nc.scalar.activation(hc, hp[:], ACT.Relu)
                scr = wp.tile([128, HPAD], BF16, tag="scr", bufs=2)
                nc.vector.tensor_tensor(out=scr[:], in0=hc, in1=p2r_t[:],
                                        op=OP.mult)
                nc.vector.tensor_reduce(out=z2[:, b:b + 1], in_=scr[:],
                                        op=OP.add, axis=AX.X)

            # masked z2 (kept-in-l1 only) -> z2ro payload -> AllGather
            zm2 = wp.tile([128, NCH], FP32, tag="zm2")
            nc.vector.tensor_tensor(out=zm2[:], in0=z2[:], in1=kp1[:],
                                    op=OP.mult)
            nc.vector.tensor_tensor(out=zm2[:], in0=zm2[:], in1=km30[:],
                                    op=OP.add)
            nc.sync.dma_start(
                z2ro[0:NPAD, :].rearrange("(b p) o -> p (b o)", p=128),
                zm2[:])
            nc.gpsimd.collective_compute(
                "AllGather", OP.bypass, replica_groups=RG,
                ins=[z2ro[:]], outs=[z2roag[:]])

            # tau2 over the masked flat payload
            ztr = wp.tile([128, NF2], FP32, tag="ztr")
            nc.sync.dma_start(
                ztr[:], z2roag[:].rearrange("(p f) o -> p (f o)", p=128))
            zt2 = wp.tile([128, NF2], FP32, tag="zt2")
            nc.vector.tensor_tensor(out=zt2[:], in0=ztr[:], in1=rom_t[:],
                                    op=OP.mult)
            rm30 = wp.tile([128, NF2], FP32, tag="rm30")
            nc.vector.tensor_scalar(rm30[:], rom_t[:], 1.0, BIG, OP.subtract,
                                    OP.mult)
            nc.vector.tensor_tensor(out=zt2[:], in0=zt2[:], in1=rm30[:],
                                    op=OP.add)
            tau2b = topk_tau(zt2, NF2, K2, "l2")

            kp2 = wp.tile([128, NCH], FP32, tag="kp2")
            nc.vector.tensor_scalar(kp2[:], zm2[:], tau2b[:, 0:1], None,
                                    OP.is_ge)
            s2 = wp.tile([128, NCH], FP32, tag="s2")
            nc.scalar.activation(s2[:], z2[:], ACT.Tanh, scale=inv2b[:, 0:1])
            a2 = wp.tile([128, NCH], FP32, tag="a2")
            nc.vector.tensor_tensor(out=a2[:], in0=s2[:], in1=kp2[:],
                                    op=OP.mult)
            km30b = wp.tile([128, NCH], FP32, tag="km30b")
            nc.vector.tensor_scalar(km30b[:], kp2[:], 1.0, BIG, OP.subtract,
                                    OP.mult)

            ro2s_ps = psS.tile([1, HPAD], FP32, tag="rosum")
            m2T = wp.tile([128, 4], FP32, tag="m2T")
            nc.vector.memset(m2T[:], -1e30)
            for b in range(NCH):
                hc = h2[:, b * HPAD:(b + 1) * HPAD]
                g2c = wp.tile([128, HPAD], BF16, tag="g1c", bufs=2)
                nc.vector.tensor_scalar(g2c[:], hc, a2[:, b:b + 1], None,
                                        OP.mult)
                nc.tensor.matmul(out=ro2s_ps[:], lhsT=onesPb_t[:], rhs=g2c[:],
                                 start=(b == 0), stop=(b == NCH - 1))
                gmc = wp.tile([128, HPAD], BF16, tag="gmc", bufs=2)
                nc.vector.tensor_scalar(gmc[:], hc, a2[:, b:b + 1],
                                        km30b[:, b:b + 1], OP.mult, OP.add)
                for fc in range(4):
                    tps = psT.tile([128, 128], BF16, tag="tps")
                    nc.tensor.transpose(out=tps[:],
                                        in_=gmc[:, fc * 128:(fc + 1) * 128],
                                        identity=idb_t[:])
                    red = wp.tile([128, 1], FP32, tag="redm", bufs=2)
                    nc.vector.tensor_reduce(out=red[:], in_=tps[:],
                                            op=OP.max, axis=AX.X)
                    nc.vector.tensor_tensor(out=m2T[:, fc:fc + 1],
                                            in0=m2T[:, fc:fc + 1],
                                            in1=red[:], op=OP.max)
            ro2s = wp.tile([1, HPAD], FP32, tag="ro2s")
            nc.vector.tensor_copy(ro2s[:], ro2s_ps[:])
            nc.sync.dma_start(ro2in[0:1, :], ro2s[:])
            nc.sync.dma_start(
                ro2in[1:2, :].rearrange("o (c p) -> p (o c)", p=128), m2T[:])
            nc.gpsimd.collective_compute(
                "AllGather", OP.bypass, replica_groups=RG,
                ins=[ro2in[:]], outs=[ro2ag[:]])

            # ======================= readout combine + head ================
            # ro1 lives in z2roag rows [s*ZRO+NPAD, s*ZRO+NPAD+1024)
            mx1 = wp.tile([128, 4], FP32, tag="mx1")
            mn1 = wp.tile([128, 4], FP32, tag="mn1")
            sums1 = wp.tile([128, 4, NCORES], FP32, tag="cmb1")
            maxs1 = wp.tile([128, 4, NCORES], FP32, tag="cmbm1")
            for s in range(NCORES):
                base = s * ZRO + NPAD
                nc.sync.dma_start(
                    sums1[:, :, s],
                    z2roag[base:base + HPAD, :]
                    .rearrange("(c p) o -> p (c o)", p=128))
                nc.sync.dma_start(
                    maxs1[:, :, s],
                    z2roag[base + HPAD:base + 1024, :]
                    .rearrange("(c p) o -> p (c o)", p=128))
            nc.vector.tensor_reduce(out=mn1[:], in_=sums1[:], op=OP.add,
                                    axis=AX.X)
            nc.vector.tensor_reduce(out=mx1[:], in_=maxs1[:], op=OP.max,
                                    axis=AX.X)
            nc.vector.tensor_scalar_mul(mn1[:], mn1[:], 1.0 / K1)

            mx2 = wp.tile([128, 4], FP32, tag="mx2")
            mn2 = wp.tile([128, 4], FP32, tag="mn2")
            sums2 = wp.tile([128, 4, 2 * NCORES], FP32, tag="cmb2")
            for r in range(2 * NCORES):
                nc.sync.dma_start(
                    sums2[:, :, r],
                    ro2ag[r:r + 1, :].rearrange("o (c p) -> p (o c)", p=128))
            s_ap = sums2[:].rearrange("p c (s t) -> p c t s", t=2)
            nc.vector.tensor_reduce(out=mn2[:], in_=s_ap[:, :, 0, :],
                                    op=OP.add, axis=AX.X)
            nc.vector.tensor_reduce(out=mx2[:], in_=s_ap[:, :, 1, :],
                                    op=OP.max, axis=AX.X)
            nc.vector.tensor_scalar_mul(mn2[:], mn2[:], 1.0 / K2)

            zT = wp.tile([128, 8], FP32, tag="zT")
            nc.vector.tensor_tensor(out=zT[:, 0:4], in0=mx1[:], in1=mx2[:],
                                    op=OP.add)
            nc.vector.tensor_tensor(out=zT[:, 4:8], in0=mn1[:], in1=mn2[:],
                                    op=OP.add)
            zTb = wp.tile([128, 8], BF16, tag="zTb")
            nc.vector.tensor_copy(zTb[:], zT[:])

            # lin1 replicated: z1h [1, 2048] via psum-row matmuls
            b1h_t = load(b1h, BF16)
            qt = [psA.tile([128, HPAD], FP32, tag="aggps", name=f"hq{q}")
                  if q < 2 else
                  psB.tile([128, HPAD], FP32, tag="hps", name=f"hq{q}")
                  for q in range(4)]
            for q in range(4):
                nc.tensor.matmul(out=qt[q][0:1, :], lhsT=ones11_t[:],
                                 rhs=b1h_t[:, q * 512:(q + 1) * 512],
                                 start=True, stop=False, skip_group_check=True)
            for t in range(8):
                l1c = sp.tile([128, 2048], BF16, tag="l1s", bufs=2)
                nc.sync.dma_start(l1c[:], l1T[t * 128:(t + 1) * 128, :])
                for q in range(4):
                    nc.tensor.matmul(
                        out=qt[q][0:1, :], lhsT=zTb[:, t:t + 1],
                        rhs=l1c[:, q * 512:(q + 1) * 512],
                        start=False, stop=(t == 7), skip_group_check=True)
            z1h = wp.tile([1, 2048], BF16, tag="z1h")
            for q in range(4):
                nc.scalar.activation(z1h[:, q * 512:(q + 1) * 512],
                                     qt[q][0:1, :], ACT.Relu)
            z1hT = wp.tile([128, 16], BF16, tag="z1hT")
            for t in range(16):
                tpv = psT.tile([128, 1], BF16, tag="tps")
                nc.tensor.transpose(out=tpv[:],
                                    in_=z1h[:, t * 128:(t + 1) * 128],
                                    identity=ones11_t[:])
                nc.scalar.activation(z1hT[:, t:t + 1], tpv[:], ACT.Copy)

            # lin2 shard (500 rows), same psum-row form
            b2h_t = load(b2h, BF16)
            o2p = psA.tile([128, HPAD], FP32, tag="aggps")
            nc.tensor.matmul(out=o2p[0:1, 0:512], lhsT=ones11_t[:],
                             rhs=b2h_t[:], start=True, stop=False,
                             skip_group_check=True)
            for t in range(16):
                l2c = sp.tile([128, 500], BF16, tag="l2s")
                nc.sync.dma_start(l2c[:], l2T[t * 128:(t + 1) * 128, :])
                nc.tensor.matmul(out=o2p[0:1, 0:500], lhsT=z1hT[:, t:t + 1],
                                 rhs=l2c[:], start=False, stop=(t == 15),
                                 skip_group_check=True)
            z2h = wp.tile([1, HPAD], BF16, tag="z2h")
            nc.vector.memset(z2h[:], 0.0)
            nc.scalar.activation(z2h[:, 0:500], o2p[0:1, 0:500], ACT.Relu)
            z2hT = wp.tile([128, 4], BF16, tag="z2hT")
            for t in range(4):
                tpv = psT.tile([128, 1], BF16, tag="tps")
                nc.tensor.transpose(out=tpv[:],
                                    in_=z2h[:, t * 128:(t + 1) * 128],
                                    identity=ones11_t[:])
                nc.scalar.activation(z2hT[:, t:t + 1], tpv[:], ACT.Copy)

            # lin3 partial (own contraction shard) + AllReduce
            l3_t = load_chunks(l3T, 4, 128, "l3Tc")
            b3h_t = load(b3h)
            o3p = psB.tile([128, 1], FP32, tag="hps")
            for t in range(4):
                nc.tensor.matmul(out=o3p[:], lhsT=l3_t[t][:],
                                 rhs=z2hT[:, t:t + 1],
                                 start=(t == 0), stop=(t == 3))
            o3 = wp.tile([128, 1], FP32, tag="o3")
            nc.vector.tensor_copy(o3[:], o3p[:])
            nc.sync.dma_start(oin[:], o3[:])
            nc.gpsimd.collective_compute(
                "AllReduce", OP.add, replica_groups=RG,
                ins=[oin[:]], outs=[oar[:]])
            fin = wp.tile([128, 1], FP32, tag="fin")
            nc.sync.dma_start(fin[:], oar[:])
            nc.scalar.activation(fin[:], fin[:], ACT.Sigmoid,
                                 bias=b3h_t[:, 0:1])
            nc.sync.dma_start(out[:].rearrange("o f -> f o"), fin[:NOUT, :])

    nc.compile()
    return nc


# ---------------------------------------------------------------------------
# entry point
# ---------------------------------------------------------------------------

_CACHE = {}
TRACE = False


def kernel(**inputs):
    prep = _pack(inputs["x"], inputs["edge_src"], inputs["edge_dst"],
                 inputs["edge_weight"])
    if "nc" not in _CACHE:
        _CACHE["nc"] = _build()
    nc = _CACHE["nc"]
    in_maps = _host_inputs(inputs, prep)
    res = bass_utils.run_bass_kernel_spmd(
        nc, in_maps, core_ids=list(range(NCORES)), trace=TRACE)
    kernel.last_results = res
    return res.results[0]["out"]


if __name__ == "__main__":
    dat = np.load("/tmp/inputs.npz")
    inputs = {k: dat[k] for k in dat.files}
    got = kernel(**inputs)
    exp = np.load("/tmp/expected.npy")
    err = np.abs(got - exp).max()
    rel = err / np.abs(exp).max()
    print("out[0,:6] =", got[0, :6])
    print("exp[0,:6] =", exp[0, :6])
    print("max abs err:", err, "rel:", rel)
